# revision 46
# baseline (speedup 1.0000x reference)
"""DeepseekV2-Lite decoder layer on 8 Trainium2 NeuronCores.

Sharding: attention is tensor-parallel over heads (2 heads/core, all tokens);
o_proj is row-parallel; the MLP is tensor-parallel over the intermediate dim
(1368 cols/core, int8 weights + runtime scales) so gate/up/down weights are
sharded 8x instead of replicated. Three collectives total: AllGather of
(x_norm^T, c_norm^T, k_pe^T, cos^T, sin^T, token-major hid), AllReduce of
o_proj partials (giving every core all-token x2 inputs for the MLP), and a
ReduceScatter of down_proj partials with x2/8 folded in so its output IS the
finished layer output. Matmuls run in bf16 with fp32 PSUM accumulation.
"""
import math
import sys

sys.path.insert(0, "/opt/trn_rl_repo")

import numpy as np
import ml_dtypes

import concourse.bass as bass
import concourse.mybir as mybir
import concourse.tile as tile
from concourse.masks import make_identity

# ---------------------------------------------------------------------------
# Patch: the hardware CTRL instruction supports only one sync-wait slot, but
# kernels with collectives need several on the final Tile drain. Split the
# excess onto SP nops emitted right after the drain, before the sem-clear.
# ---------------------------------------------------------------------------
from concourse.vector_clock import ScopedClock


def _drain_and_barrier_split(self, tick_clock, wait_clock):
    drain_inst = self.nc.sync.drain()
    wait_clock.add_sem_waits(
        drain_inst.ins, ScopedClock({None: tick_clock.global_clock})
    )
    si = drain_inst.ins.sync_info
    if si is not None and len(si.on_wait) > 1:
        waits = list(si.on_wait)
        drain_inst.ins.sync_info = mybir.SyncInfo(
            on_wait=waits[:1], on_update=list(si.on_update)
        )
        for w in waits[1:]:
            nop = self.nc.sync.nop(nofuse=True, hint="drain_wait_overflow")
            nop.ins.sync_info = mybir.SyncInfo(on_wait=[w], on_update=[])
    self.nc.all_engine_barrier()
    assert self.sems is not None
    popped = self.nc._tile_sem_poison_stack.pop()
    assert popped is self._sem_poison
    self.nc.clear_and_free_semaphores(list(self.sems.allocated().values()))
    self.nc.all_engine_barrier()


tile.TileContext._drain_and_barrier = _drain_and_barrier_split

# ---------------------------------------------------------------------------
# Several instruction encodings (DMA, CTRL) accept only one sync-wait slot.
# Split every multi-wait instruction at BIR-serialization time: excess waits
# move onto same-engine NoOps inserted immediately before the instruction.
# ---------------------------------------------------------------------------
import orjson as _orjson

if not getattr(bass.Bass, "_wait_split_patched", False):
    bass.Bass._orig_to_json_bytes = bass.Bass.to_json_bytes
    bass.Bass._wait_split_patched = True
_orig_to_json_bytes = bass.Bass._orig_to_json_bytes


def _to_json_bytes_split(self):
    data = _orjson.loads(_orig_to_json_bytes(self))
    ctr = 0
    for f in data.get("functions", []):
        for bb in f.get("basic_blocks", f.get("blocks", [])):
            insts = bb.get("instructions", [])
            out = []
            for inst in insts:
                si = inst.get("sync_info")
                if si and len(si.get("on_wait") or []) > 1:
                    waits = si["on_wait"]
                    for w in waits[:-1]:
                        ctr += 1
                        out.append({
                            "debug": inst.get("debug", 0),
                            "engine": inst["engine"],
                            "ins": [], "name": f"I-ws{ctr}",
                            "opcode": "NoOp", "outs": [],
                            "sync_info": {"on_update": [], "on_wait": [w]},
                            "text_hint": "wait_split",
                        })
                    si["on_wait"] = [waits[-1]]
                out.append(inst)
            bb["instructions"] = out
    return _orjson.dumps(data)


bass.Bass.to_json_bytes = _to_json_bytes_split

# ---------------------------------------------------------------------------
FULL_CFG = dict(
    B=2, S=2048, HID=2048, H=16, D_NOPE=128, D_ROPE=64, D_V=128, KV=512,
    INTER=10944, N_CORES=8,
)
EPS = 1e-6
MAX_POS, BASE, FACTOR, ORIG_MAX = 8192, 10000.0, 40.0, 4096
BETA_FAST, BETA_SLOW, MSCALE, MSCALE_ALL = 32, 1, 0.707, 0.707

BF = mybir.dt.bfloat16
F32 = mybir.dt.float32
I8 = mybir.dt.int8
F16 = mybir.dt.float16
AX = mybir.AxisListType
AF = mybir.ActivationFunctionType


def _derived(cfg):
    d = dict(cfg)
    d["T_TOT"] = cfg["B"] * cfg["S"]
    d["T_LOC"] = d["T_TOT"] // cfg["N_CORES"]
    d["HPC"] = cfg["H"] // cfg["N_CORES"]
    d["KH"] = cfg["HID"] // 128
    d["KC"] = cfg["KV"] // 128
    d["TSUB"] = d["T_LOC"] // 128
    d["NCH"] = d["T_TOT"] // d["T_LOC"]
    d["ILOC"] = cfg["INTER"] // cfg["N_CORES"]   # 1368 intermediate cols/core
    d["ICL"] = (d["ILOC"] + 127) // 128          # 11 padded k-tiles/core
    d["IPAD"] = d["ICL"] * 128                   # 1408
    d["QTILES_B"] = cfg["S"] // 512
    d["KB_B"] = cfg["S"] // 128
    d["DQ"] = cfg["D_NOPE"] + cfg["D_ROPE"]
    # xnT + cnT + kpeT + cosT + sinT + flat token-major hid
    d["AGROWS"] = 2 * cfg["HID"] + cfg["KV"] + 2 * cfg["D_ROPE"]
    return d


# ---------------------------------------------------------------------------
def build_kernel(cfg):
    c = _derived(cfg)
    N = c["N_CORES"]
    HID, KV, DR, DN, DV = c["HID"], c["KV"], c["D_ROPE"], c["D_NOPE"], c["D_V"]
    TL, TT = c["T_LOC"], c["T_TOT"]
    KH, KC, TSUB, NCH, ICL = c["KH"], c["KC"], c["TSUB"], c["NCH"], c["ICL"]
    HPC, DQ = c["HPC"], c["DQ"]
    QT_B, KB_B = c["QTILES_B"], c["KB_B"]
    B = c["B"]
    HR = DR // 2
    AGR = c["AGROWS"]

    nc = bass.Bass()
    hid_e = nc.dram_tensor("hid", [TL, HID], BF, kind="ExternalInput")
    wqT_e = nc.dram_tensor("wqT", [HID, HPC * DQ], BF, kind="ExternalInput")
    wkvaT_e = nc.dram_tensor("wkvaT", [HID, KV + DR], BF, kind="ExternalInput")
    wbnT_e = nc.dram_tensor("wbnT", [KV, HPC * DN], BF, kind="ExternalInput")
    wbvT_e = nc.dram_tensor("wbvT", [KV, HPC * DV], BF, kind="ExternalInput")
    woT_e = nc.dram_tensor("woT", [HPC * DV, HID], BF, kind="ExternalInput")
    wg_e = nc.dram_tensor("wg3", [ICL, 128, KH, 128], I8, kind="ExternalInput")
    wu_e = nc.dram_tensor("wu3", [ICL, 128, KH, 128], I8, kind="ExternalInput")
    wd_e = nc.dram_tensor("wd3", [ICL, 128, HID], I8, kind="ExternalInput")
    sg_e = nc.dram_tensor("sg", [128, ICL], F32, kind="ExternalInput")
    su_e = nc.dram_tensor("su", [128, ICL], F32, kind="ExternalInput")
    sd_e = nc.dram_tensor("sd", [128, ICL], F32, kind="ExternalInput")
    cosL_e = nc.dram_tensor("cosL", [TL, HR], F32, kind="ExternalInput")
    sinL_e = nc.dram_tensor("sinL", [TL, HR], F32, kind="ExternalInput")
    out_e = nc.dram_tensor("out", [TL, HID], F16, kind="ExternalOutput")
    probe = cfg.get("probe", False)
    if probe:
        p_agin_e = nc.dram_tensor("p_agin", [AGR, TL], BF, kind="ExternalOutput")

    with tile.TileContext(nc) as tc:
        with (
            tc.tile_pool(name="dram", bufs=1, space="DRAM") as dram,
            tc.tile_pool(name="const", bufs=1) as const,
        ):
            agin = dram.tile([AGR, TL], BF, tag="agin", name="agin")
            agout = dram.tile([N * AGR, TL], BF, addr_space="Shared", tag="agout", name="agout")
            rs_in = dram.tile([TT, HID], F16, tag="rsin", name="rsin")
            x2a = dram.tile([TT, HID], F16, addr_space="Shared", tag="x2a", name="x2a")
            rs2_in = dram.tile([TT, HID], F32, tag="rs2in", name="rs2in")
            rs2_out = dram.tile([TL, HID], F32, tag="rs2out", name="rs2out")
            OFF_KPE = HID + KV
            OFF_COS = OFF_KPE + DR
            OFF_SIN = OFF_COS + HR
            OFF_HID = OFF_SIN + HR

            ident = const.tile([128, 128], BF, tag="ident", name="ident")
            make_identity(nc, ident)
            eps_sb = const.tile([128, 1], F32, tag="eps", name="eps")
            nc.vector.memset(eps_sb[:], EPS)
            # mask[p, x] = 1.0 where x >= p + 384, else 0 — generated on device
            mask_sb = const.tile([128, 896], BF, tag="mask", name="mask")
            nc.gpsimd.memset(mask_sb[:], 1.0)
            nc.gpsimd.affine_select(
                out=mask_sb[:], in_=mask_sb[:],
                compare_op=mybir.AluOpType.is_ge, fill=0.0,
                base=-384, pattern=[[1, 896]], channel_multiplier=-1)
            cosL_sb = const.tile([128, TSUB, HR], F32, tag="cosL", name="cosL")
            nc.sync.dma_start(cosL_sb[:], cosL_e.rearrange("(a p) r -> p a r", p=128))
            sinL_sb = const.tile([128, TSUB, HR], F32, tag="sinL", name="sinL")
            nc.sync.dma_start(sinL_sb[:], sinL_e.rearrange("(a p) r -> p a r", p=128))
            sg_sb = const.tile([128, ICL], F32, tag="sg", name="sg")
            nc.sync.dma_start(sg_sb[:], sg_e[:])
            su_sb = const.tile([128, ICL], F32, tag="su", name="su")
            nc.sync.dma_start(su_sb[:], su_e[:])
            sd_sb = const.tile([128, ICL], F32, tag="sd", name="sd")
            nc.sync.dma_start(sd_sb[:], sd_e[:])

            # ============ phases 0-1: rms1, x^T, ckv, rms(c), rope(k_pe) =====
            with (
                tc.tile_pool(name="xnTp", bufs=1) as xnTp,
                tc.tile_pool(name="p0", bufs=2) as p0,
                tc.tile_pool(name="p01ps", bufs=2, space="PSUM") as p01ps,
            ):
                xnT = [xnTp.tile([128, TL], BF, tag=f"xnT{k}", name=f"xnT{k}") for k in range(KH)]
                # token-major hid rides the AllGather as a flat [TL*HID/TL, TL] region
                nc.sync.dma_start(
                    agin[OFF_HID:OFF_HID + HID, :],
                    hid_e.rearrange("t (a c) -> (t a) c", c=TL))
                xn_sb = []
                for t in range(TSUB):
                    ht = p0.tile([128, HID], BF, tag="hid0", name="hid0")
                    nc.sync.dma_start(ht[:], hid_e[t * 128:(t + 1) * 128, :])
                    sq = p0.tile([128, HID], F32, tag="sq", name="sq")
                    nc.vector.tensor_mul(sq[:], ht[:], ht[:])
                    ssum = p0.tile([128, 1], F32, tag="ssum", name="ssum")
                    nc.vector.reduce_sum(out=ssum[:], in_=sq[:], axis=AX.X)
                    rs = p0.tile([128, 1], F32, tag="rs", name="rs")
                    nc.scalar.activation(rs[:], ssum[:], AF.Sqrt, scale=1.0 / HID, bias=eps_sb[:])
                    nc.vector.reciprocal(rs[:], rs[:])
                    xt = p0.tile([128, HID], BF, tag="xn", name="xn", bufs=TSUB)
                    nc.vector.tensor_scalar_mul(xt[:], ht[:], rs[:])
                    xn_sb.append(xt)
                for t in range(TSUB):
                    for k in range(KH):
                        ps = p01ps.tile([128, 128], BF, tag="tr", name="tr")
                        nc.tensor.transpose(ps[:], xn_sb[t][:, k * 128:(k + 1) * 128], ident[:])
                        nc.scalar.copy(xnT[k][:, t * 128:(t + 1) * 128], ps[:])
                for k in range(KH):
                    nc.sync.dma_start(agin[k * 128:(k + 1) * 128, :], xnT[k][:])

                # phase 1
                wkva_sb = [p0.tile([128, KV + DR], BF, tag=f"wkva{k}", name=f"wkva{k}") for k in range(KH)]
                for k in range(KH):
                    nc.sync.dma_start(wkva_sb[k][:], wkvaT_e[k * 128:(k + 1) * 128, :])
                cnT_sb = [p0.tile([128, TL], BF, tag=f"cnT{j}", name=f"cnT{j}") for j in range(KC)]
                kpeT_loc = p0.tile([DR, TL], BF, tag="kpeT_loc", name="kpeT_loc")
                for t in range(TSUB):
                    ps_c = p01ps.tile([128, KV], F32, tag="psc", name="psc")
                    ps_p = p01ps.tile([128, DR], F32, tag="psp", name="psp")
                    for k in range(KH):
                        lq = xnT[k][:, t * 128:(t + 1) * 128]
                        nc.tensor.matmul(ps_c[:], lq, wkva_sb[k][:, :KV],
                                         start=(k == 0), stop=(k == KH - 1))
                        nc.tensor.matmul(ps_p[:], lq, wkva_sb[k][:, KV:],
                                         start=(k == 0), stop=(k == KH - 1))
                    sq = p0.tile([128, KV], F32, tag="sqc", name="sqc")
                    nc.scalar.activation(sq[:], ps_c[:], AF.Square)
                    ssum = p0.tile([128, 1], F32, tag="ssumc", name="ssumc")
                    nc.vector.reduce_sum(out=ssum[:], in_=sq[:], axis=AX.X)
                    rs = p0.tile([128, 1], F32, tag="rsc", name="rsc")
                    nc.scalar.activation(rs[:], ssum[:], AF.Sqrt, scale=1.0 / KV, bias=eps_sb[:])
                    nc.vector.reciprocal(rs[:], rs[:])
                    cn = p0.tile([128, KV], BF, tag="cn", name="cn")
                    nc.vector.tensor_scalar_mul(cn[:], ps_c[:], rs[:])
                    kp = p0.tile([128, DR], BF, tag="kp", name="kp")
                    a = p0.tile([128, HR], F32, tag="ra", name="ra")
                    b = p0.tile([128, HR], F32, tag="rb", name="rb")
                    cosl = cosL_sb[:, t, :]
                    sinl = sinL_sb[:, t, :]
                    nc.vector.tensor_mul(a[:], ps_p[:, :HR], cosl)
                    nc.vector.tensor_mul(b[:], ps_p[:, HR:], sinl)
                    nc.vector.tensor_sub(kp[:, :HR], a[:], b[:])
                    nc.vector.tensor_mul(a[:], ps_p[:, HR:], cosl)
                    nc.vector.tensor_mul(b[:], ps_p[:, :HR], sinl)
                    nc.vector.tensor_add(kp[:, HR:], a[:], b[:])
                    for j in range(KC):
                        ps = p01ps.tile([128, 128], BF, tag="tr", name="tr")
                        nc.tensor.transpose(ps[:], cn[:, j * 128:(j + 1) * 128], ident[:])
                        nc.scalar.copy(cnT_sb[j][:, t * 128:(t + 1) * 128], ps[:])
                    ps = p01ps.tile([128, 128], BF, tag="tr", name="tr")
                    nc.tensor.transpose(ps[:DR, :], kp[:], ident[:])
                    nc.scalar.copy(kpeT_loc[:, t * 128:(t + 1) * 128], ps[:DR, :])
                for j in range(KC):
                    nc.sync.dma_start(agin[HID + j * 128:HID + (j + 1) * 128, :], cnT_sb[j][:])
                nc.sync.dma_start(agin[OFF_KPE:OFF_KPE + DR, :], kpeT_loc[:])
                # ride local cos/sin (transposed, bf16) for the q-rope phase
                cl_bf = p0.tile([128, TSUB, HR], BF, tag="clbf", name="clbf", bufs=1)
                nc.scalar.copy(cl_bf[:], cosL_sb[:])
                sl_bf = p0.tile([128, TSUB, HR], BF, tag="slbf", name="slbf", bufs=1)
                nc.scalar.copy(sl_bf[:], sinL_sb[:])
                cosT_loc = p0.tile([HR, TL], BF, tag="cosTl", name="cosTl", bufs=1)
                sinT_loc = p0.tile([HR, TL], BF, tag="sinTl", name="sinTl", bufs=1)
                for t in range(TSUB):
                    ps = p01ps.tile([128, 128], BF, tag="tr", name="tr")
                    nc.tensor.transpose(ps[:HR, :], cl_bf[:, t, :], ident[:])
                    nc.scalar.copy(cosT_loc[:, t * 128:(t + 1) * 128], ps[:HR, :])
                    ps = p01ps.tile([128, 128], BF, tag="tr", name="tr")
                    nc.tensor.transpose(ps[:HR, :], sl_bf[:, t, :], ident[:])
                    nc.scalar.copy(sinT_loc[:, t * 128:(t + 1) * 128], ps[:HR, :])
                nc.sync.dma_start(agin[OFF_COS:OFF_COS + HR, :], cosT_loc[:])
                nc.sync.dma_start(agin[OFF_SIN:OFF_SIN + HR, :], sinT_loc[:])

            # ============ phase 2: AllGather ================================
            nc.gpsimd.collective_compute(
                "AllGather", mybir.AluOpType.bypass,
                replica_groups=[list(range(N))],
                ins=[agin.opt()], outs=[agout.opt()],
            )

            if probe:
                with tc.tile_pool(name="prb0", bufs=2) as prb0:
                    for r in range(0, AGR, 128):
                        w = min(128, AGR - r)
                        pt_ = prb0.tile([128, TL], BF, tag="pgt", name="pgt")
                        nc.sync.dma_start(pt_[:w, :], agin[r:r + w, :])
                        nc.sync.dma_start(p_agin_e[r:r + w, :], pt_[:w, :])

            # ============ phases 3-5: attention ==============================
            with tc.tile_pool(name="asb", bufs=1) as asb:
                qnT = [asb.tile([128, TT], BF, tag=f"qnT{h}", name=f"qnT{h}") for h in range(HPC)]
                qpT = [asb.tile([DR, TT], BF, tag=f"qpT{h}", name=f"qpT{h}") for h in range(HPC)]
                knT = [asb.tile([128, TT], BF, tag=f"knT{h}", name=f"knT{h}") for h in range(HPC)]
                kpeT = asb.tile([DR, TT], BF, tag="kpeT", name="kpeT")
                v_sb = [asb.tile([128, TT // 128, DV + 4], BF, tag=f"v{h}", name=f"v{h}")
                        for h in range(HPC)]
                atT = [asb.tile([128, TT], BF, tag=f"atT{h}", name=f"atT{h}") for h in range(HPC)]
                cosT_sb = asb.tile([HR, TT], BF, tag="cosT", name="cosT")
                sinT_sb = asb.tile([HR, TT], BF, tag="sinT", name="sinT")

                with (
                    tc.tile_pool(name="p4w", bufs=1) as p4w,
                    tc.tile_pool(name="p4x", bufs=1) as p4x,
                    tc.tile_pool(name="p4", bufs=2) as p4,
                    tc.tile_pool(name="p4ps", bufs=2, space="PSUM") as p4ps,
                ):
                    wq_sb = [p4w.tile([128, HPC * DQ], BF, tag=f"wq{k}", name=f"wq{k}") for k in range(KH)]
                    for k in range(KH):
                        nc.sync.dma_start(wq_sb[k][:], wqT_e[k * 128:(k + 1) * 128, :])
                    wbn_sb = [p4w.tile([128, HPC * DN], BF, tag=f"wbn{j}", name=f"wbn{j}") for j in range(KC)]
                    wbv_sb = [p4w.tile([128, HPC * DV], BF, tag=f"wbv{j}", name=f"wbv{j}") for j in range(KC)]
                    for j in range(KC):
                        nc.sync.dma_start(wbn_sb[j][:], wbnT_e[j * 128:(j + 1) * 128, :])
                        nc.sync.dma_start(wbv_sb[j][:], wbvT_e[j * 128:(j + 1) * 128, :])

                    for ch in range(NCH):
                        nc.sync.dma_start(
                            kpeT[:, ch * TL:(ch + 1) * TL],
                            agout[ch * AGR + OFF_KPE: ch * AGR + OFF_KPE + DR, :])
                        nc.sync.dma_start(
                            cosT_sb[:, ch * TL:(ch + 1) * TL],
                            agout[ch * AGR + OFF_COS: ch * AGR + OFF_COS + HR, :])
                        nc.sync.dma_start(
                            sinT_sb[:, ch * TL:(ch + 1) * TL],
                            agout[ch * AGR + OFF_SIN: ch * AGR + OFF_SIN + HR, :])

                    for ch in range(NCH):
                        xch = []
                        for k in range(KH):
                            xt = p4x.tile([128, TL], BF, tag="xch", name="xch", bufs=KH + 4)
                            nc.sync.dma_start(
                                xt[:], agout[ch * AGR + k * 128: ch * AGR + (k + 1) * 128, :])
                            xch.append(xt)
                        cs = slice(ch * TL, (ch + 1) * TL)
                        for h in range(HPC):
                            ps_n = p4ps.tile([128, TL], F32, tag="qn", name="qn")
                            ps_p = p4ps.tile([DR, TL], F32, tag="qp", name="qp")
                            off = h * DQ
                            for k in range(KH):
                                nc.tensor.matmul(ps_n[:], wq_sb[k][:, off:off + DN], xch[k][:],
                                                 start=(k == 0), stop=(k == KH - 1))
                            for k in range(KH):
                                nc.tensor.matmul(ps_p[:], wq_sb[k][:, off + DN:off + DQ], xch[k][:],
                                                 start=(k == 0), stop=(k == KH - 1))
                            nc.scalar.copy(qnT[h][:, cs], ps_n[:])
                            a = p4.tile([HR, TL], F32, tag="qa", name="qa")
                            b = p4.tile([HR, TL], F32, tag="qb", name="qb")
                            cosc = cosT_sb[:, cs]
                            sinc = sinT_sb[:, cs]
                            nc.vector.tensor_mul(a[:], ps_p[:HR, :], cosc)
                            nc.vector.tensor_mul(b[:], ps_p[HR:, :], sinc)
                            nc.vector.tensor_sub(qpT[h][:HR, cs], a[:], b[:])
                            nc.vector.tensor_mul(a[:], ps_p[HR:, :], cosc)
                            nc.vector.tensor_mul(b[:], ps_p[:HR, :], sinc)
                            nc.vector.tensor_add(qpT[h][HR:, cs], a[:], b[:])

                    for ch in range(NCH):
                        cch = []
                        for j in range(KC):
                            ct = p4x.tile([128, TL], BF, tag="cch", name="cch", bufs=KC + 2)
                            nc.sync.dma_start(
                                ct[:], agout[ch * AGR + HID + j * 128: ch * AGR + HID + (j + 1) * 128, :])
                            cch.append(ct)
                        cs = slice(ch * TL, (ch + 1) * TL)
                        for h in range(HPC):
                            ps_k = p4ps.tile([128, TL], F32, tag="kn", name="kn")
                            for j in range(KC):
                                nc.tensor.matmul(ps_k[:], wbn_sb[j][:, h * DN:(h + 1) * DN], cch[j][:],
                                                 start=(j == 0), stop=(j == KC - 1))
                            nc.scalar.copy(knT[h][:, cs], ps_k[:])
                            for j4 in range(TL // 128):
                                ps_v = p4ps.tile([128, DV], F32, tag="pv", name="pv")
                                for j in range(KC):
                                    nc.tensor.matmul(ps_v[:], cch[j][:, j4 * 128:(j4 + 1) * 128],
                                                     wbv_sb[j][:, h * DV:(h + 1) * DV],
                                                     start=(j == 0), stop=(j == KC - 1))
                                kbt = ch * (TL // 128) + j4
                                nc.scalar.copy(v_sb[h][:, kbt, :DV], ps_v[:])
                                nc.vector.memset(v_sb[h][:, kbt, DV:DV + 1], 1.0)

                # ---------------- phase 5: attention -------------------------
                with (
                    tc.tile_pool(name="p5ps", bufs=2, space="PSUM") as p5ps,
                    tc.tile_pool(name="p5pv", bufs=2, space="PSUM") as p5pv,
                    tc.tile_pool(name="p5", bufs=2) as p5,
                    tc.tile_pool(name="prb", bufs=1) as prb,
                ):
                    for b in range(B):
                        for h in range(HPC):
                            for qt in range(QT_B):
                                qs = slice(b * cfg["S"] + qt * 512, b * cfg["S"] + qt * 512 + 512)
                                nkb = 4 * qt + 4
                                pt = []
                                for kb in range(nkb):
                                    kbg = b * KB_B + kb
                                    ks = slice(kbg * 128, kbg * 128 + 128)
                                    ps_s = p5ps.tile([128, 512], F32, tag="ps_s", name="ps_s")
                                    nc.tensor.matmul(ps_s[:], knT[h][:, ks], qnT[h][:, qs],
                                                     start=True, stop=False)
                                    nc.tensor.matmul(ps_s[:], kpeT[:, ks], qpT[h][:, qs],
                                                     start=False, stop=True)
                                    pb = prb.tile([128, 512], BF, tag="pb", name="pb", bufs=KB_B + 4)
                                    nc.scalar.activation(pb[:], ps_s[:], AF.Exp)
                                    delta = kb * 128 - qt * 512
                                    if delta >= 0:
                                        nc.vector.tensor_mul(
                                            pb[:], pb[:], mask_sb[:, 384 - delta:896 - delta])
                                    pt.append(pb)
                                for q4 in range(4):
                                    ps_av = p5pv.tile([128, DV + 4], F32, tag="ps_av", name="ps_av")
                                    for kb in range(nkb):
                                        kbt = b * KB_B + kb
                                        nc.tensor.matmul(
                                            ps_av[:, :DV + 1],
                                            pt[kb][:, q4 * 128:(q4 + 1) * 128],
                                            v_sb[h][:, kbt, :DV + 1],
                                            start=(kb == 0), stop=(kb == nkb - 1))
                                    recip = p5.tile([128, 1], F32, tag="recip", name="recip")
                                    nc.vector.reciprocal(recip[:], ps_av[:, DV:DV + 1])
                                    at = p5.tile([128, DV], BF, tag="at", name="at")
                                    nc.vector.tensor_scalar_mul(at[:], ps_av[:, :DV], recip[:])
                                    ps_t = p5ps.tile([128, 128], BF, tag="ps_t", name="ps_t")
                                    nc.tensor.transpose(ps_t[:DV, :], at[:], ident[:])
                                    qg = (b * cfg["S"] + qt * 512) // 128 + q4
                                    nc.scalar.copy(atT[h][:DV, qg * 128:(qg + 1) * 128], ps_t[:DV, :])

                # ============ phase 5b: row-parallel o_proj partials =============
                with (
                    tc.tile_pool(name="p6w", bufs=1) as p6w,
                    tc.tile_pool(name="p6", bufs=4) as p6,
                    tc.tile_pool(name="p6ps", bufs=4, space="PSUM") as p6ps,
                ):
                    wo_sb = [p6w.tile([128, HID], BF, tag=f"wo{j}", name=f"wo{j}") for j in range(HPC)]
                    for j in range(HPC):
                        nc.sync.dma_start(wo_sb[j][:], woT_e[j * DV:(j + 1) * DV, :])
                    for tq in range(TT // 128):
                        for nsl in range(HID // 512):
                            ps_o = p6ps.tile([128, 512], F32, tag="ps_o", name="ps_o")
                            for j in range(HPC):
                                nc.tensor.matmul(ps_o[:], atT[j][:DV, tq * 128:(tq + 1) * 128],
                                                 wo_sb[j][:, nsl * 512:(nsl + 1) * 512],
                                                 start=(j == 0), stop=(j == HPC - 1))
                            ob = p6.tile([128, 512], F16, tag="ob", name="ob")
                            nc.scalar.copy(ob[:], ps_o[:])
                            nc.sync.dma_start(
                                rs_in[tq * 128:(tq + 1) * 128, nsl * 512:(nsl + 1) * 512], ob[:])

            # ============ phase 6: AllReduce o_proj partials ================
            nc.gpsimd.collective_compute(
                "AllReduce", mybir.AluOpType.add,
                replica_groups=[list(range(N))],
                ins=[rs_in.opt()], outs=[x2a.opt()],
            )

            # ============ phases 7-8: x2, rms2, TP MLP over INTER ============
            # Every core: for each 512-token chunk, assemble x2 = o_attn + hid
            # (both all-token), rms2 + transpose to y^T, gate/up/down on its
            # 1368-col INTER slice, fold x2/8 into the down partials so the
            # final ReduceScatter(add) emits the finished layer output.
            with (
                tc.tile_pool(name="p8wd", bufs=1) as p8wd,
                tc.tile_pool(name="p8w", bufs=2) as p8w,
                tc.tile_pool(name="p8x", bufs=2) as p8x,
                tc.tile_pool(name="p8sq", bufs=2) as p8sq,
                tc.tile_pool(name="p8y", bufs=1) as p8y,
                tc.tile_pool(name="p8h", bufs=1) as p8h,
                tc.tile_pool(name="p8", bufs=4) as p8,
                tc.tile_pool(name="p8ps", bufs=2, space="PSUM") as p8ps,
                tc.tile_pool(name="p8psd", bufs=2, space="PSUM") as p8psd,
                tc.tile_pool(name="p8pst", bufs=2, space="PSUM") as p8pst,
            ):
                wd_sb = [p8wd.tile([128, HID], BF, tag=f"wd{i}", name=f"wd{i}")
                         for i in range(ICL)]
                for i in range(ICL):
                    wdq = p8w.tile([128, HID], I8, tag="wdq", name="wdq")
                    nc.sync.dma_start(wdq[:], wd_e[i])
                    nc.vector.tensor_scalar_mul(wd_sb[i][:], wdq[:], sd_sb[:, i:i + 1])
                for ch in range(NCH):
                    x2c, x2s = [], []
                    for t in range(TSUB):
                        oc = p8x.tile([128, HID], F16, tag="oc", name="oc")
                        nc.sync.dma_start(
                            oc[:], x2a[ch * TL + t * 128: ch * TL + (t + 1) * 128, :])
                        hc = p8x.tile([128, HID], BF, tag="hc", name="hc")
                        nc.sync.dma_start(
                            hc[:],
                            agout[ch * AGR + OFF_HID + t * 512:
                                  ch * AGR + OFF_HID + (t + 1) * 512, :]
                            .rearrange("(p a) c -> p (a c)", p=128))
                        xb = p8x.tile([128, HID], F16, tag="xc", name="xc", bufs=TSUB)
                        nc.vector.tensor_add(xb[:], oc[:], hc[:])
                        x2c.append(xb)
                        xs = p8x.tile([128, HID], F16, tag="xs", name="xs", bufs=TSUB)
                        nc.scalar.activation(xs[:], xb[:], AF.Copy, scale=0.125)
                        x2s.append(xs)
                    ynT = [p8y.tile([128, TL], BF, tag=f"ynT{k}", name=f"ynT{k}", bufs=1)
                           for k in range(KH)]
                    for t in range(TSUB):
                        sq = p8sq.tile([128, HID], F32, tag="sq", name="sq")
                        nc.vector.tensor_mul(sq[:], x2c[t][:], x2c[t][:])
                        ssum = p8.tile([128, 1], F32, tag="ssum", name="ssum")
                        nc.vector.reduce_sum(out=ssum[:], in_=sq[:], axis=AX.X)
                        rsc = p8.tile([128, 1], F32, tag="rsc", name="rsc")
                        nc.scalar.activation(rsc[:], ssum[:], AF.Sqrt, scale=1.0 / HID, bias=eps_sb[:])
                        nc.vector.reciprocal(rsc[:], rsc[:])
                        yt = p8.tile([128, HID], BF, tag="yn", name="yn", bufs=2)
                        nc.vector.tensor_scalar_mul(yt[:], x2c[t][:], rsc[:])
                        for k in range(KH):
                            ps = p8pst.tile([128, 128], BF, tag="tr", name="tr")
                            nc.tensor.transpose(ps[:], yt[:, k * 128:(k + 1) * 128], ident[:])
                            nc.scalar.copy(ynT[k][:, t * 128:(t + 1) * 128], ps[:])
                    hT = []
                    for i in range(ICL):
                        wgq = p8w.tile([128, KH, 128], I8, tag="wgq", name="wgq")
                        nc.sync.dma_start(wgq[:], wg_e[i])
                        wg_sb = p8w.tile([128, KH, 128], BF, tag="wg", name="wg")
                        nc.scalar.copy(wg_sb[:], wgq[:])
                        wuq = p8w.tile([128, KH, 128], I8, tag="wuq", name="wuq")
                        nc.sync.dma_start(wuq[:], wu_e[i])
                        wu_sb = p8w.tile([128, KH, 128], BF, tag="wu", name="wu")
                        nc.scalar.copy(wu_sb[:], wuq[:])
                        ps_g = p8ps.tile([128, TL], F32, tag="psg", name="psg")
                        ps_u = p8ps.tile([128, TL], F32, tag="psu", name="psu")
                        for k in range(KH):
                            nc.tensor.matmul(ps_g[:], wg_sb[:, k, :], ynT[k][:],
                                             start=(k == 0), stop=(k == KH - 1))
                        for k in range(KH):
                            nc.tensor.matmul(ps_u[:], wu_sb[:, k, :], ynT[k][:],
                                             start=(k == 0), stop=(k == KH - 1))
                        sig = p8.tile([128, TL], BF, tag="sig", name="sig")
                        nc.scalar.activation(sig[:], ps_g[:], AF.Silu,
                                             scale=sg_sb[:, i:i + 1])
                        ub = p8.tile([128, TL], BF, tag="ub", name="ub")
                        nc.vector.tensor_scalar_mul(ub[:], ps_u[:], su_sb[:, i:i + 1])
                        ht = p8h.tile([128, TL], BF, tag="hT", name="hT", bufs=ICL + 2)
                        nc.vector.tensor_mul(ht[:], sig[:], ub[:])
                        hT.append(ht)
                    for tt in range(TSUB):
                        for ng in range(HID // 512):
                            ps_d = p8psd.tile([128, 512], F32, tag="psd", name="psd")
                            for i in range(ICL):
                                nc.tensor.matmul(ps_d[:], hT[i][:, tt * 128:(tt + 1) * 128],
                                                 wd_sb[i][:, ng * 512:(ng + 1) * 512],
                                                 start=(i == 0), stop=(i == ICL - 1))
                            ob = p8.tile([128, 512], F32, tag="ob", name="ob")
                            nc.vector.tensor_add(
                                ob[:], ps_d[:], x2s[tt][:, ng * 512:(ng + 1) * 512])
                            nc.sync.dma_start(
                                rs2_in[ch * TL + tt * 128: ch * TL + (tt + 1) * 128,
                                       ng * 512:(ng + 1) * 512], ob[:])

            # ============ phase 9: ReduceScatter -> finished output ==========
            nc.gpsimd.collective_compute(
                "ReduceScatter", mybir.AluOpType.add,
                replica_groups=[list(range(N))],
                ins=[rs2_in.opt()], outs=[rs2_out.opt()],
            )
            with tc.tile_pool(name="p9", bufs=4) as p9:
                for t in range(TSUB):
                    fin = p9.tile([128, HID], F32, tag="fin", name="fin")
                    nc.sync.dma_start(fin[:], rs2_out[t * 128:(t + 1) * 128, :])
                    fb = p9.tile([128, HID], F16, tag="fb", name="fb")
                    nc.scalar.copy(fb[:], fin[:])
                    nc.sync.dma_start(out_e[t * 128:(t + 1) * 128, :], fb[:])
    return nc


# ---------------------------------------------------------------------------
# Host-side prep
# ---------------------------------------------------------------------------
def _yarn_tables(position_ids, d_rope):
    ar = np.arange(0, d_rope, 2, dtype=np.float32) / d_rope
    freq_extra = 1.0 / BASE ** ar
    freq_inter = 1.0 / (FACTOR * BASE ** ar)

    def corr_dim(num_rot):
        return d_rope * math.log(ORIG_MAX / (num_rot * 2 * math.pi)) / (2 * math.log(BASE))

    low = max(math.floor(corr_dim(BETA_FAST)), 0)
    high = min(math.ceil(corr_dim(BETA_SLOW)), d_rope - 1)
    hi = high + 0.001 if low == high else high
    ramp = np.clip((np.arange(d_rope // 2, dtype=np.float32) - low) / (hi - low), 0.0, 1.0)
    inv_freq_mask = 1.0 - ramp
    inv_freq = freq_inter * (1 - inv_freq_mask) + freq_extra * inv_freq_mask

    def get_mscale(s, m):
        return 1.0 if s <= 1 else 0.1 * m * math.log(s) + 1.0

    ms = get_mscale(FACTOR, MSCALE) / get_mscale(FACTOR, MSCALE_ALL)
    pos = np.asarray(position_ids).reshape(-1).astype(np.float32)
    fr = np.outer(pos, inv_freq)
    return (np.cos(fr) * ms).astype(np.float32), (np.sin(fr) * ms).astype(np.float32)


def _deint_perm(d):
    p = np.empty(d, np.int64)
    p[:d // 2] = 2 * np.arange(d // 2)
    p[d // 2:] = 2 * np.arange(d // 2) + 1
    return p


def prep_inputs(cfg, hidden_states, position_ids, Wq, Wkva, w_kvln, Wkvb, Wo,
                Wg, Wu, Wd, w_ln1, w_ln2):
    c = _derived(cfg)
    N, HPC = c["N_CORES"], c["HPC"]
    HID, KV, DR, DN, DV, DQ = c["HID"], c["KV"], c["D_ROPE"], c["D_NOPE"], c["D_V"], c["DQ"]
    TL, TT, KH = c["T_LOC"], c["T_TOT"], c["KH"]
    ILOC, ICL, IPAD = c["ILOC"], c["ICL"], c["IPAD"]
    bf = ml_dtypes.bfloat16

    hid_flat = np.ascontiguousarray(hidden_states.reshape(TT, HID)).astype(bf)
    perm = _deint_perm(DR)
    scale = np.float32(DQ ** -0.5)

    Wq = Wq * w_ln1[None, :] * scale
    Wqh = Wq.reshape(cfg["H"], DQ, HID)
    Wqh = np.concatenate([Wqh[:, :DN], Wqh[:, DN:][:, perm]], axis=1)
    Wkva = Wkva * w_ln1[None, :]
    Wkva = np.concatenate([Wkva[:KV], Wkva[KV:][perm]], axis=0)
    wkvaT = np.ascontiguousarray(Wkva.T).astype(bf)
    Wkvb = Wkvb * w_kvln[None, :]
    Wkvbh = Wkvb.reshape(cfg["H"], DN + DV, KV)
    WoT_f = np.ascontiguousarray(Wo.T, dtype=np.float32)
    WgT_f = (Wg * w_ln2[None, :]).T          # [HID, INTER]
    WuT_f = (Wu * w_ln2[None, :]).T
    WdT_f = Wd.T                             # [INTER, HID]

    def _quant_cols(w):
        # per-column symmetric int8: w[:, i] = q[:, i] * s[i]
        s = np.abs(w).max(axis=0) / 127.0
        s[s == 0] = 1.0
        q = np.clip(np.round(w / s[None, :]), -127, 127).astype(np.int8)
        return q, s.astype(np.float32)

    def _mlp_slices(core):
        i0 = core * ILOC
        gc = np.zeros((HID, IPAD), np.float32)
        gc[:, :ILOC] = WgT_f[:, i0:i0 + ILOC]
        uc = np.zeros((HID, IPAD), np.float32)
        uc[:, :ILOC] = WuT_f[:, i0:i0 + ILOC]
        dc = np.zeros((IPAD, HID), np.float32)
        dc[:ILOC] = WdT_f[i0:i0 + ILOC]
        gq, sg = _quant_cols(gc)
        uq, su = _quant_cols(uc)
        dqT, sd = _quant_cols(dc.T)                          # per-row of dc
        dq = np.ascontiguousarray(dqT.T)
        wg3 = np.ascontiguousarray(gq.reshape(KH, 128, ICL, 128).transpose(2, 1, 0, 3))
        wu3 = np.ascontiguousarray(uq.reshape(KH, 128, ICL, 128).transpose(2, 1, 0, 3))
        wd3 = np.ascontiguousarray(dq.reshape(ICL, 128, HID))
        sg2 = np.ascontiguousarray(sg.reshape(ICL, 128).T)   # [i_inner, i_tile]
        su2 = np.ascontiguousarray(su.reshape(ICL, 128).T)
        sd2 = np.ascontiguousarray(sd.reshape(ICL, 128).T)
        return wg3, wu3, wd3, sg2, su2, sd2

    cos_f, sin_f = _yarn_tables(position_ids, DR)

    in_maps = []
    for core in range(N):
        h0 = core * HPC
        wqT = np.ascontiguousarray(
            Wqh[h0:h0 + HPC].transpose(2, 0, 1).reshape(HID, HPC * DQ)).astype(bf)
        wbnT = np.ascontiguousarray(
            Wkvbh[h0:h0 + HPC, :DN].transpose(2, 0, 1).reshape(KV, HPC * DN)).astype(bf)
        wbvT = np.ascontiguousarray(
            Wkvbh[h0:h0 + HPC, DN:].transpose(2, 0, 1).reshape(KV, HPC * DV)).astype(bf)
        wg3, wu3, wd3, sg2, su2, sd2 = _mlp_slices(core)
        sl = slice(core * TL, (core + 1) * TL)
        in_maps.append({
            "hid": hid_flat[sl],
            "wqT": wqT,
            "wkvaT": wkvaT,
            "wbnT": wbnT,
            "wbvT": wbvT,
            "woT": np.ascontiguousarray(WoT_f[h0 * DV:(h0 + HPC) * DV]).astype(bf),
            "wg3": wg3,
            "wu3": wu3,
            "wd3": wd3,
            "sg": sg2,
            "su": su2,
            "sd": sd2,
            "cosL": np.ascontiguousarray(cos_f[sl]),
            "sinL": np.ascontiguousarray(sin_f[sl]),
        })
    return in_maps


def run_cfg(cfg, nc, inputs_dict):
    from concourse.bass_utils import run_bass_kernel_spmd
    c = _derived(cfg)
    in_maps = prep_inputs(cfg, **inputs_dict)
    res = run_bass_kernel_spmd(nc, in_maps, list(range(cfg["N_CORES"])))
    out = np.concatenate(
        [res.results[i]["out"] for i in range(cfg["N_CORES"])], axis=0)
    return out.reshape(cfg["B"], cfg["S"], cfg["HID"]).astype(np.float32), res


_NC_CACHE = {}


def kernel(hidden_states, position_ids, Wq, Wkva, w_kvln, Wkvb, Wo, Wg, Wu, Wd,
           w_ln1, w_ln2):
    cfg = FULL_CFG
    if "full" not in _NC_CACHE:
        _NC_CACHE["full"] = build_kernel(cfg)
    out, _ = run_cfg(cfg, _NC_CACHE["full"], dict(
        hidden_states=np.asarray(hidden_states, np.float32),
        position_ids=np.asarray(position_ids),
        Wq=np.asarray(Wq, np.float32), Wkva=np.asarray(Wkva, np.float32),
        w_kvln=np.asarray(w_kvln, np.float32), Wkvb=np.asarray(Wkvb, np.float32),
        Wo=np.asarray(Wo, np.float32), Wg=np.asarray(Wg, np.float32),
        Wu=np.asarray(Wu, np.float32), Wd=np.asarray(Wd, np.float32),
        w_ln1=np.asarray(w_ln1, np.float32), w_ln2=np.asarray(w_ln2, np.float32)))
    return out



# revision 48
# speedup vs baseline: 1.0710x; 1.0710x over previous
"""DeepseekV2-Lite decoder layer on 8 Trainium2 NeuronCores.

Sharding: attention is tensor-parallel over heads (2 heads/core, all tokens);
o_proj is row-parallel; the MLP is tensor-parallel over the intermediate dim
(1368 cols/core, int8 weights + runtime scales) so gate/up/down weights are
sharded 8x instead of replicated. Three collectives total: AllGather of
(x_norm^T, c_norm^T, k_pe^T, cos^T, sin^T, token-major hid), AllReduce of
o_proj partials (giving every core all-token x2 inputs for the MLP), and a
ReduceScatter of down_proj partials with x2/8 folded in so its output IS the
finished layer output. Matmuls run in bf16 with fp32 PSUM accumulation.
"""
import math
import sys

sys.path.insert(0, "/opt/trn_rl_repo")

import numpy as np
import ml_dtypes

import concourse.bass as bass
import concourse.mybir as mybir
import concourse.tile as tile
from concourse.masks import make_identity

# ---------------------------------------------------------------------------
# Patch: the hardware CTRL instruction supports only one sync-wait slot, but
# kernels with collectives need several on the final Tile drain. Split the
# excess onto SP nops emitted right after the drain, before the sem-clear.
# ---------------------------------------------------------------------------
from concourse.vector_clock import ScopedClock


def _drain_and_barrier_split(self, tick_clock, wait_clock):
    drain_inst = self.nc.sync.drain()
    wait_clock.add_sem_waits(
        drain_inst.ins, ScopedClock({None: tick_clock.global_clock})
    )
    si = drain_inst.ins.sync_info
    if si is not None and len(si.on_wait) > 1:
        waits = list(si.on_wait)
        drain_inst.ins.sync_info = mybir.SyncInfo(
            on_wait=waits[:1], on_update=list(si.on_update)
        )
        for w in waits[1:]:
            nop = self.nc.sync.nop(nofuse=True, hint="drain_wait_overflow")
            nop.ins.sync_info = mybir.SyncInfo(on_wait=[w], on_update=[])
    self.nc.all_engine_barrier()
    assert self.sems is not None
    popped = self.nc._tile_sem_poison_stack.pop()
    assert popped is self._sem_poison
    self.nc.clear_and_free_semaphores(list(self.sems.allocated().values()))
    self.nc.all_engine_barrier()


tile.TileContext._drain_and_barrier = _drain_and_barrier_split

# ---------------------------------------------------------------------------
# Several instruction encodings (DMA, CTRL) accept only one sync-wait slot.
# Split every multi-wait instruction at BIR-serialization time: excess waits
# move onto same-engine NoOps inserted immediately before the instruction.
# ---------------------------------------------------------------------------
import orjson as _orjson

if not getattr(bass.Bass, "_wait_split_patched", False):
    bass.Bass._orig_to_json_bytes = bass.Bass.to_json_bytes
    bass.Bass._wait_split_patched = True
_orig_to_json_bytes = bass.Bass._orig_to_json_bytes


def _to_json_bytes_split(self):
    data = _orjson.loads(_orig_to_json_bytes(self))
    ctr = 0
    for f in data.get("functions", []):
        for bb in f.get("basic_blocks", f.get("blocks", [])):
            insts = bb.get("instructions", [])
            out = []
            for inst in insts:
                si = inst.get("sync_info")
                if si and len(si.get("on_wait") or []) > 1:
                    waits = si["on_wait"]
                    for w in waits[:-1]:
                        ctr += 1
                        out.append({
                            "debug": inst.get("debug", 0),
                            "engine": inst["engine"],
                            "ins": [], "name": f"I-ws{ctr}",
                            "opcode": "NoOp", "outs": [],
                            "sync_info": {"on_update": [], "on_wait": [w]},
                            "text_hint": "wait_split",
                        })
                    si["on_wait"] = [waits[-1]]
                out.append(inst)
            bb["instructions"] = out
    return _orjson.dumps(data)


bass.Bass.to_json_bytes = _to_json_bytes_split

# ---------------------------------------------------------------------------
FULL_CFG = dict(
    B=2, S=2048, HID=2048, H=16, D_NOPE=128, D_ROPE=64, D_V=128, KV=512,
    INTER=10944, N_CORES=8,
)
EPS = 1e-6
MAX_POS, BASE, FACTOR, ORIG_MAX = 8192, 10000.0, 40.0, 4096
BETA_FAST, BETA_SLOW, MSCALE, MSCALE_ALL = 32, 1, 0.707, 0.707

BF = mybir.dt.bfloat16
F32 = mybir.dt.float32
I8 = mybir.dt.int8
F16 = mybir.dt.float16
AX = mybir.AxisListType
AF = mybir.ActivationFunctionType


def _derived(cfg):
    d = dict(cfg)
    d["T_TOT"] = cfg["B"] * cfg["S"]
    d["T_LOC"] = d["T_TOT"] // cfg["N_CORES"]
    d["HPC"] = cfg["H"] // cfg["N_CORES"]
    d["KH"] = cfg["HID"] // 128
    d["KC"] = cfg["KV"] // 128
    d["TSUB"] = d["T_LOC"] // 128
    d["NCH"] = d["T_TOT"] // d["T_LOC"]
    d["ILOC"] = cfg["INTER"] // cfg["N_CORES"]   # 1368 intermediate cols/core
    d["ICL"] = (d["ILOC"] + 127) // 128          # 11 padded k-tiles/core
    d["IPAD"] = d["ICL"] * 128                   # 1408
    d["QTILES_B"] = cfg["S"] // 512
    d["KB_B"] = cfg["S"] // 128
    d["DQ"] = cfg["D_NOPE"] + cfg["D_ROPE"]
    # xnT + cnT + kpeT + cosT + sinT + flat token-major hid
    d["AGROWS"] = 2 * cfg["HID"] + cfg["KV"] + 2 * cfg["D_ROPE"]
    return d


# ---------------------------------------------------------------------------
def build_kernel(cfg):
    c = _derived(cfg)
    N = c["N_CORES"]
    HID, KV, DR, DN, DV = c["HID"], c["KV"], c["D_ROPE"], c["D_NOPE"], c["D_V"]
    TL, TT = c["T_LOC"], c["T_TOT"]
    KH, KC, TSUB, NCH, ICL = c["KH"], c["KC"], c["TSUB"], c["NCH"], c["ICL"]
    HPC, DQ = c["HPC"], c["DQ"]
    QT_B, KB_B = c["QTILES_B"], c["KB_B"]
    B = c["B"]
    HR = DR // 2
    AGR = c["AGROWS"]

    nc = bass.Bass()
    hid_e = nc.dram_tensor("hid", [TL, HID], BF, kind="ExternalInput")
    wqT_e = nc.dram_tensor("wqT", [HID, HPC * DQ], I8, kind="ExternalInput")
    wkvaT_e = nc.dram_tensor("wkvaT", [HID, KV + DR], I8, kind="ExternalInput")
    wbnT_e = nc.dram_tensor("wbnT", [KV, HPC * DN], BF, kind="ExternalInput")
    wbvT_e = nc.dram_tensor("wbvT", [KV, HPC * DV], BF, kind="ExternalInput")
    woT_e = nc.dram_tensor("woT", [HPC * DV, HID], I8, kind="ExternalInput")
    wg_e = nc.dram_tensor("wg3", [ICL, 128, KH, 128], I8, kind="ExternalInput")
    wu_e = nc.dram_tensor("wu3", [ICL, 128, KH, 128], I8, kind="ExternalInput")
    wd_e = nc.dram_tensor("wd3", [ICL, 128, HID], I8, kind="ExternalInput")
    satt_e = nc.dram_tensor("satt", [128, 3], F32, kind="ExternalInput")
    sg_e = nc.dram_tensor("sg", [128, ICL], F32, kind="ExternalInput")
    su_e = nc.dram_tensor("su", [128, ICL], F32, kind="ExternalInput")
    sd_e = nc.dram_tensor("sd", [128, ICL], F32, kind="ExternalInput")
    cosL_e = nc.dram_tensor("cosL", [TL, HR], F32, kind="ExternalInput")
    sinL_e = nc.dram_tensor("sinL", [TL, HR], F32, kind="ExternalInput")
    out_e = nc.dram_tensor("out", [TL, HID], F16, kind="ExternalOutput")
    probe = cfg.get("probe", False)
    if probe:
        p_agin_e = nc.dram_tensor("p_agin", [AGR, TL], BF, kind="ExternalOutput")

    with tile.TileContext(nc) as tc:
        with (
            tc.tile_pool(name="dram", bufs=1, space="DRAM") as dram,
            tc.tile_pool(name="const", bufs=1) as const,
        ):
            agin = dram.tile([AGR, TL], BF, tag="agin", name="agin")
            agout = dram.tile([N * AGR, TL], BF, addr_space="Shared", tag="agout", name="agout")
            rs_in = dram.tile([TT, HID], F16, tag="rsin", name="rsin")
            x2a = dram.tile([TT, HID], F16, addr_space="Shared", tag="x2a", name="x2a")
            rs2_in = dram.tile([TT, HID], F32, tag="rs2in", name="rs2in")
            rs2_out = dram.tile([TL, HID], F32, tag="rs2out", name="rs2out")
            OFF_KPE = HID + KV
            OFF_COS = OFF_KPE + DR
            OFF_SIN = OFF_COS + HR
            OFF_HID = OFF_SIN + HR

            ident = const.tile([128, 128], BF, tag="ident", name="ident")
            make_identity(nc, ident)
            eps_sb = const.tile([128, 1], F32, tag="eps", name="eps")
            nc.vector.memset(eps_sb[:], EPS)
            # mask[p, x] = 1.0 where x >= p + 384, else 0 — generated on device
            mask_sb = const.tile([128, 896], BF, tag="mask", name="mask")
            nc.gpsimd.memset(mask_sb[:], 1.0)
            nc.gpsimd.affine_select(
                out=mask_sb[:], in_=mask_sb[:],
                compare_op=mybir.AluOpType.is_ge, fill=0.0,
                base=-384, pattern=[[1, 896]], channel_multiplier=-1)
            cosL_sb = const.tile([128, TSUB, HR], F32, tag="cosL", name="cosL")
            nc.sync.dma_start(cosL_sb[:], cosL_e.rearrange("(a p) r -> p a r", p=128))
            sinL_sb = const.tile([128, TSUB, HR], F32, tag="sinL", name="sinL")
            nc.sync.dma_start(sinL_sb[:], sinL_e.rearrange("(a p) r -> p a r", p=128))
            satt_sb = const.tile([128, 3], F32, tag="satt", name="satt")
            nc.sync.dma_start(satt_sb[:], satt_e[:])
            sg_sb = const.tile([128, ICL], F32, tag="sg", name="sg")
            nc.sync.dma_start(sg_sb[:], sg_e[:])
            su_sb = const.tile([128, ICL], F32, tag="su", name="su")
            nc.sync.dma_start(su_sb[:], su_e[:])
            sd_sb = const.tile([128, ICL], F32, tag="sd", name="sd")
            nc.sync.dma_start(sd_sb[:], sd_e[:])

            # ============ phases 0-1: rms1, x^T, ckv, rms(c), rope(k_pe) =====
            with (
                tc.tile_pool(name="xnTp", bufs=1) as xnTp,
                tc.tile_pool(name="p0", bufs=2) as p0,
                tc.tile_pool(name="p01ps", bufs=2, space="PSUM") as p01ps,
            ):
                xnT = [xnTp.tile([128, TL], BF, tag=f"xnT{k}", name=f"xnT{k}") for k in range(KH)]
                # token-major hid rides the AllGather as a flat [TL*HID/TL, TL] region
                nc.sync.dma_start(
                    agin[OFF_HID:OFF_HID + HID, :],
                    hid_e.rearrange("t (a c) -> (t a) c", c=TL))
                xn_sb = []
                for t in range(TSUB):
                    ht = p0.tile([128, HID], BF, tag="hid0", name="hid0")
                    nc.sync.dma_start(ht[:], hid_e[t * 128:(t + 1) * 128, :])
                    sq = p0.tile([128, HID], F32, tag="sq", name="sq")
                    nc.vector.tensor_mul(sq[:], ht[:], ht[:])
                    ssum = p0.tile([128, 1], F32, tag="ssum", name="ssum")
                    nc.vector.reduce_sum(out=ssum[:], in_=sq[:], axis=AX.X)
                    rs = p0.tile([128, 1], F32, tag="rs", name="rs")
                    nc.scalar.activation(rs[:], ssum[:], AF.Sqrt, scale=1.0 / HID, bias=eps_sb[:])
                    nc.vector.reciprocal(rs[:], rs[:])
                    xt = p0.tile([128, HID], BF, tag="xn", name="xn", bufs=TSUB)
                    nc.vector.tensor_scalar_mul(xt[:], ht[:], rs[:])
                    xn_sb.append(xt)
                for t in range(TSUB):
                    for k in range(KH):
                        ps = p01ps.tile([128, 128], BF, tag="tr", name="tr")
                        nc.tensor.transpose(ps[:], xn_sb[t][:, k * 128:(k + 1) * 128], ident[:])
                        nc.scalar.copy(xnT[k][:, t * 128:(t + 1) * 128], ps[:])
                for k in range(KH):
                    nc.sync.dma_start(agin[k * 128:(k + 1) * 128, :], xnT[k][:])

                # phase 1
                wkva_sb = [p0.tile([128, KV + DR], BF, tag=f"wkva{k}", name=f"wkva{k}") for k in range(KH)]
                for k in range(KH):
                    wkq = p0.tile([128, KV + DR], I8, tag="wkq", name="wkq")
                    nc.sync.dma_start(wkq[:], wkvaT_e[k * 128:(k + 1) * 128, :])
                    nc.scalar.copy(wkva_sb[k][:], wkq[:])
                # cos/sin scaled by s_kv for the k_pe rope (k_pe psum is raw int units)
                clk = p0.tile([128, TSUB, HR], F32, tag="clk", name="clk", bufs=1)
                nc.vector.tensor_scalar_mul(clk[:], cosL_sb[:], satt_sb[:, 1:2])
                slk = p0.tile([128, TSUB, HR], F32, tag="slk", name="slk", bufs=1)
                nc.vector.tensor_scalar_mul(slk[:], sinL_sb[:], satt_sb[:, 1:2])
                cnT_sb = [p0.tile([128, TL], BF, tag=f"cnT{j}", name=f"cnT{j}") for j in range(KC)]
                kpeT_loc = p0.tile([DR, TL], BF, tag="kpeT_loc", name="kpeT_loc")
                for t in range(TSUB):
                    ps_c = p01ps.tile([128, KV], F32, tag="psc", name="psc")
                    ps_p = p01ps.tile([128, DR], F32, tag="psp", name="psp")
                    for k in range(KH):
                        lq = xnT[k][:, t * 128:(t + 1) * 128]
                        nc.tensor.matmul(ps_c[:], lq, wkva_sb[k][:, :KV],
                                         start=(k == 0), stop=(k == KH - 1))
                        nc.tensor.matmul(ps_p[:], lq, wkva_sb[k][:, KV:],
                                         start=(k == 0), stop=(k == KH - 1))
                    sq = p0.tile([128, KV], F32, tag="sqc", name="sqc")
                    nc.scalar.activation(sq[:], ps_c[:], AF.Square)
                    ssum = p0.tile([128, 1], F32, tag="ssumc", name="ssumc")
                    nc.vector.reduce_sum(out=ssum[:], in_=sq[:], axis=AX.X)
                    rs = p0.tile([128, 1], F32, tag="rsc", name="rsc")
                    nc.scalar.activation(rs[:], ssum[:], AF.Sqrt, scale=1.0 / KV, bias=eps_sb[:])
                    nc.vector.reciprocal(rs[:], rs[:])
                    cn = p0.tile([128, KV], BF, tag="cn", name="cn")
                    nc.vector.tensor_scalar_mul(cn[:], ps_c[:], rs[:])
                    kp = p0.tile([128, DR], BF, tag="kp", name="kp")
                    a = p0.tile([128, HR], F32, tag="ra", name="ra")
                    b = p0.tile([128, HR], F32, tag="rb", name="rb")
                    cosl = clk[:, t, :]
                    sinl = slk[:, t, :]
                    nc.vector.tensor_mul(a[:], ps_p[:, :HR], cosl)
                    nc.vector.tensor_mul(b[:], ps_p[:, HR:], sinl)
                    nc.vector.tensor_sub(kp[:, :HR], a[:], b[:])
                    nc.vector.tensor_mul(a[:], ps_p[:, HR:], cosl)
                    nc.vector.tensor_mul(b[:], ps_p[:, :HR], sinl)
                    nc.vector.tensor_add(kp[:, HR:], a[:], b[:])
                    for j in range(KC):
                        ps = p01ps.tile([128, 128], BF, tag="tr", name="tr")
                        nc.tensor.transpose(ps[:], cn[:, j * 128:(j + 1) * 128], ident[:])
                        nc.scalar.copy(cnT_sb[j][:, t * 128:(t + 1) * 128], ps[:])
                    ps = p01ps.tile([128, 128], BF, tag="tr", name="tr")
                    nc.tensor.transpose(ps[:DR, :], kp[:], ident[:])
                    nc.scalar.copy(kpeT_loc[:, t * 128:(t + 1) * 128], ps[:DR, :])
                for j in range(KC):
                    nc.sync.dma_start(agin[HID + j * 128:HID + (j + 1) * 128, :], cnT_sb[j][:])
                nc.sync.dma_start(agin[OFF_KPE:OFF_KPE + DR, :], kpeT_loc[:])
                # ride local cos/sin (transposed, bf16) for the q-rope phase
                cl_bf = p0.tile([128, TSUB, HR], BF, tag="clbf", name="clbf", bufs=1)
                nc.vector.tensor_scalar_mul(cl_bf[:], cosL_sb[:], satt_sb[:, 0:1])
                sl_bf = p0.tile([128, TSUB, HR], BF, tag="slbf", name="slbf", bufs=1)
                nc.vector.tensor_scalar_mul(sl_bf[:], sinL_sb[:], satt_sb[:, 0:1])
                cosT_loc = p0.tile([HR, TL], BF, tag="cosTl", name="cosTl", bufs=1)
                sinT_loc = p0.tile([HR, TL], BF, tag="sinTl", name="sinTl", bufs=1)
                for t in range(TSUB):
                    ps = p01ps.tile([128, 128], BF, tag="tr", name="tr")
                    nc.tensor.transpose(ps[:HR, :], cl_bf[:, t, :], ident[:])
                    nc.scalar.copy(cosT_loc[:, t * 128:(t + 1) * 128], ps[:HR, :])
                    ps = p01ps.tile([128, 128], BF, tag="tr", name="tr")
                    nc.tensor.transpose(ps[:HR, :], sl_bf[:, t, :], ident[:])
                    nc.scalar.copy(sinT_loc[:, t * 128:(t + 1) * 128], ps[:HR, :])
                nc.sync.dma_start(agin[OFF_COS:OFF_COS + HR, :], cosT_loc[:])
                nc.sync.dma_start(agin[OFF_SIN:OFF_SIN + HR, :], sinT_loc[:])

            # ============ phase 2: AllGather ================================
            nc.gpsimd.collective_compute(
                "AllGather", mybir.AluOpType.bypass,
                replica_groups=[list(range(N))],
                ins=[agin.opt()], outs=[agout.opt()],
            )

            if probe:
                with tc.tile_pool(name="prb0", bufs=2) as prb0:
                    for r in range(0, AGR, 128):
                        w = min(128, AGR - r)
                        pt_ = prb0.tile([128, TL], BF, tag="pgt", name="pgt")
                        nc.sync.dma_start(pt_[:w, :], agin[r:r + w, :])
                        nc.sync.dma_start(p_agin_e[r:r + w, :], pt_[:w, :])

            # ============ phases 3-5: attention ==============================
            with tc.tile_pool(name="asb", bufs=1) as asb:
                qnT = [asb.tile([128, TT], BF, tag=f"qnT{h}", name=f"qnT{h}") for h in range(HPC)]
                qpT = [asb.tile([DR, TT], BF, tag=f"qpT{h}", name=f"qpT{h}") for h in range(HPC)]
                knT = [asb.tile([128, TT], BF, tag=f"knT{h}", name=f"knT{h}") for h in range(HPC)]
                kpeT = asb.tile([DR, TT], BF, tag="kpeT", name="kpeT")
                v_sb = [asb.tile([128, TT // 128, DV + 4], BF, tag=f"v{h}", name=f"v{h}")
                        for h in range(HPC)]
                atT = [asb.tile([128, TT], BF, tag=f"atT{h}", name=f"atT{h}") for h in range(HPC)]
                cosT_sb = asb.tile([HR, TT], BF, tag="cosT", name="cosT")
                sinT_sb = asb.tile([HR, TT], BF, tag="sinT", name="sinT")

                with (
                    tc.tile_pool(name="p4w", bufs=1) as p4w,
                    tc.tile_pool(name="p4x", bufs=1) as p4x,
                    tc.tile_pool(name="p4", bufs=2) as p4,
                    tc.tile_pool(name="p4ps", bufs=2, space="PSUM") as p4ps,
                ):
                    wq_sb = [p4w.tile([128, HPC * DQ], BF, tag=f"wq{k}", name=f"wq{k}") for k in range(KH)]
                    for k in range(KH):
                        wqq = p4.tile([128, HPC * DQ], I8, tag="wqq", name="wqq")
                        nc.sync.dma_start(wqq[:], wqT_e[k * 128:(k + 1) * 128, :])
                        nc.scalar.copy(wq_sb[k][:], wqq[:])
                    wbn_sb = [p4w.tile([128, HPC * DN], BF, tag=f"wbn{j}", name=f"wbn{j}") for j in range(KC)]
                    wbv_sb = [p4w.tile([128, HPC * DV], BF, tag=f"wbv{j}", name=f"wbv{j}") for j in range(KC)]
                    for j in range(KC):
                        nc.sync.dma_start(wbn_sb[j][:], wbnT_e[j * 128:(j + 1) * 128, :])
                        nc.sync.dma_start(wbv_sb[j][:], wbvT_e[j * 128:(j + 1) * 128, :])

                    for ch in range(NCH):
                        nc.sync.dma_start(
                            kpeT[:, ch * TL:(ch + 1) * TL],
                            agout[ch * AGR + OFF_KPE: ch * AGR + OFF_KPE + DR, :])
                        nc.sync.dma_start(
                            cosT_sb[:, ch * TL:(ch + 1) * TL],
                            agout[ch * AGR + OFF_COS: ch * AGR + OFF_COS + HR, :])
                        nc.sync.dma_start(
                            sinT_sb[:, ch * TL:(ch + 1) * TL],
                            agout[ch * AGR + OFF_SIN: ch * AGR + OFF_SIN + HR, :])

                    for ch in range(NCH):
                        xch = []
                        for k in range(KH):
                            xt = p4x.tile([128, TL], BF, tag="xch", name="xch", bufs=KH + 4)
                            nc.sync.dma_start(
                                xt[:], agout[ch * AGR + k * 128: ch * AGR + (k + 1) * 128, :])
                            xch.append(xt)
                        cs = slice(ch * TL, (ch + 1) * TL)
                        for h in range(HPC):
                            ps_n = p4ps.tile([128, TL], F32, tag="qn", name="qn")
                            ps_p = p4ps.tile([DR, TL], F32, tag="qp", name="qp")
                            off = h * DQ
                            for k in range(KH):
                                nc.tensor.matmul(ps_n[:], wq_sb[k][:, off:off + DN], xch[k][:],
                                                 start=(k == 0), stop=(k == KH - 1))
                            for k in range(KH):
                                nc.tensor.matmul(ps_p[:], wq_sb[k][:, off + DN:off + DQ], xch[k][:],
                                                 start=(k == 0), stop=(k == KH - 1))
                            nc.vector.tensor_scalar_mul(qnT[h][:, cs], ps_n[:], satt_sb[:, 0:1])
                            a = p4.tile([HR, TL], F32, tag="qa", name="qa")
                            b = p4.tile([HR, TL], F32, tag="qb", name="qb")
                            cosc = cosT_sb[:, cs]
                            sinc = sinT_sb[:, cs]
                            nc.vector.tensor_mul(a[:], ps_p[:HR, :], cosc)
                            nc.vector.tensor_mul(b[:], ps_p[HR:, :], sinc)
                            nc.vector.tensor_sub(qpT[h][:HR, cs], a[:], b[:])
                            nc.vector.tensor_mul(a[:], ps_p[HR:, :], cosc)
                            nc.vector.tensor_mul(b[:], ps_p[:HR, :], sinc)
                            nc.vector.tensor_add(qpT[h][HR:, cs], a[:], b[:])

                    for ch in range(NCH):
                        cch = []
                        for j in range(KC):
                            ct = p4x.tile([128, TL], BF, tag="cch", name="cch", bufs=KC + 2)
                            nc.sync.dma_start(
                                ct[:], agout[ch * AGR + HID + j * 128: ch * AGR + HID + (j + 1) * 128, :])
                            cch.append(ct)
                        cs = slice(ch * TL, (ch + 1) * TL)
                        for h in range(HPC):
                            ps_k = p4ps.tile([128, TL], F32, tag="kn", name="kn")
                            for j in range(KC):
                                nc.tensor.matmul(ps_k[:], wbn_sb[j][:, h * DN:(h + 1) * DN], cch[j][:],
                                                 start=(j == 0), stop=(j == KC - 1))
                            nc.scalar.copy(knT[h][:, cs], ps_k[:])
                            for j4 in range(TL // 128):
                                ps_v = p4ps.tile([128, DV], F32, tag="pv", name="pv")
                                for j in range(KC):
                                    nc.tensor.matmul(ps_v[:], cch[j][:, j4 * 128:(j4 + 1) * 128],
                                                     wbv_sb[j][:, h * DV:(h + 1) * DV],
                                                     start=(j == 0), stop=(j == KC - 1))
                                kbt = ch * (TL // 128) + j4
                                nc.scalar.copy(v_sb[h][:, kbt, :DV], ps_v[:])
                                nc.vector.memset(v_sb[h][:, kbt, DV:DV + 1], 1.0)

                # ---------------- phase 5: attention -------------------------
                with (
                    tc.tile_pool(name="p5ps", bufs=2, space="PSUM") as p5ps,
                    tc.tile_pool(name="p5pv", bufs=2, space="PSUM") as p5pv,
                    tc.tile_pool(name="p5", bufs=2) as p5,
                    tc.tile_pool(name="prb", bufs=1) as prb,
                ):
                    for b in range(B):
                        for h in range(HPC):
                            for qt in range(QT_B):
                                qs = slice(b * cfg["S"] + qt * 512, b * cfg["S"] + qt * 512 + 512)
                                nkb = 4 * qt + 4
                                pt = []
                                for kb in range(nkb):
                                    kbg = b * KB_B + kb
                                    ks = slice(kbg * 128, kbg * 128 + 128)
                                    ps_s = p5ps.tile([128, 512], F32, tag="ps_s", name="ps_s")
                                    nc.tensor.matmul(ps_s[:], knT[h][:, ks], qnT[h][:, qs],
                                                     start=True, stop=False)
                                    nc.tensor.matmul(ps_s[:], kpeT[:, ks], qpT[h][:, qs],
                                                     start=False, stop=True)
                                    pb = prb.tile([128, 512], BF, tag="pb", name="pb", bufs=KB_B + 4)
                                    nc.scalar.activation(pb[:], ps_s[:], AF.Exp)
                                    delta = kb * 128 - qt * 512
                                    if delta >= 0:
                                        nc.vector.tensor_mul(
                                            pb[:], pb[:], mask_sb[:, 384 - delta:896 - delta])
                                    pt.append(pb)
                                for q4 in range(4):
                                    ps_av = p5pv.tile([128, DV + 4], F32, tag="ps_av", name="ps_av")
                                    for kb in range(nkb):
                                        kbt = b * KB_B + kb
                                        nc.tensor.matmul(
                                            ps_av[:, :DV + 1],
                                            pt[kb][:, q4 * 128:(q4 + 1) * 128],
                                            v_sb[h][:, kbt, :DV + 1],
                                            start=(kb == 0), stop=(kb == nkb - 1))
                                    recip = p5.tile([128, 1], F32, tag="recip", name="recip")
                                    nc.vector.reciprocal(recip[:], ps_av[:, DV:DV + 1])
                                    at = p5.tile([128, DV], BF, tag="at", name="at")
                                    nc.vector.tensor_scalar_mul(at[:], ps_av[:, :DV], recip[:])
                                    ps_t = p5ps.tile([128, 128], BF, tag="ps_t", name="ps_t")
                                    nc.tensor.transpose(ps_t[:DV, :], at[:], ident[:])
                                    qg = (b * cfg["S"] + qt * 512) // 128 + q4
                                    nc.scalar.copy(atT[h][:DV, qg * 128:(qg + 1) * 128], ps_t[:DV, :])

                # ============ phase 5b: row-parallel o_proj partials =============
                with (
                    tc.tile_pool(name="p6w", bufs=1) as p6w,
                    tc.tile_pool(name="p6", bufs=4) as p6,
                    tc.tile_pool(name="p6ps", bufs=4, space="PSUM") as p6ps,
                ):
                    wo_sb = [p6w.tile([128, HID], BF, tag=f"wo{j}", name=f"wo{j}") for j in range(HPC)]
                    for j in range(HPC):
                        woq = p6.tile([128, HID], I8, tag="woq", name="woq")
                        nc.sync.dma_start(woq[:DV, :], woT_e[j * DV:(j + 1) * DV, :])
                        nc.scalar.copy(wo_sb[j][:DV, :], woq[:DV, :])
                    for tq in range(TT // 128):
                        for nsl in range(HID // 512):
                            ps_o = p6ps.tile([128, 512], F32, tag="ps_o", name="ps_o")
                            for j in range(HPC):
                                nc.tensor.matmul(ps_o[:], atT[j][:DV, tq * 128:(tq + 1) * 128],
                                                 wo_sb[j][:, nsl * 512:(nsl + 1) * 512],
                                                 start=(j == 0), stop=(j == HPC - 1))
                            ob = p6.tile([128, 512], F16, tag="ob", name="ob")
                            nc.vector.tensor_scalar_mul(ob[:], ps_o[:], satt_sb[:, 2:3])
                            nc.sync.dma_start(
                                rs_in[tq * 128:(tq + 1) * 128, nsl * 512:(nsl + 1) * 512], ob[:])

            # ============ phase 6: AllReduce o_proj partials ================
            nc.gpsimd.collective_compute(
                "AllReduce", mybir.AluOpType.add,
                replica_groups=[list(range(N))],
                ins=[rs_in.opt()], outs=[x2a.opt()],
            )

            # ============ phases 7-8: x2, rms2, TP MLP over INTER ============
            # Every core: for each 512-token chunk, assemble x2 = o_attn + hid
            # (both all-token), rms2 + transpose to y^T, gate/up/down on its
            # 1368-col INTER slice, fold x2/8 into the down partials so the
            # final ReduceScatter(add) emits the finished layer output.
            with (
                tc.tile_pool(name="p8wd", bufs=1) as p8wd,
                tc.tile_pool(name="p8w", bufs=2) as p8w,
                tc.tile_pool(name="p8x", bufs=2) as p8x,
                tc.tile_pool(name="p8sq", bufs=2) as p8sq,
                tc.tile_pool(name="p8y", bufs=1) as p8y,
                tc.tile_pool(name="p8h", bufs=1) as p8h,
                tc.tile_pool(name="p8", bufs=4) as p8,
                tc.tile_pool(name="p8ps", bufs=2, space="PSUM") as p8ps,
                tc.tile_pool(name="p8psd", bufs=2, space="PSUM") as p8psd,
                tc.tile_pool(name="p8pst", bufs=2, space="PSUM") as p8pst,
            ):
                wd_sb = [p8wd.tile([128, HID], BF, tag=f"wd{i}", name=f"wd{i}")
                         for i in range(ICL)]
                for i in range(ICL):
                    wdq = p8w.tile([128, HID], I8, tag="wdq", name="wdq")
                    nc.sync.dma_start(wdq[:], wd_e[i])
                    nc.vector.tensor_scalar_mul(wd_sb[i][:], wdq[:], sd_sb[:, i:i + 1])
                for ch in range(NCH):
                    x2c, x2s = [], []
                    for t in range(TSUB):
                        oc = p8x.tile([128, HID], F16, tag="oc", name="oc")
                        nc.sync.dma_start(
                            oc[:], x2a[ch * TL + t * 128: ch * TL + (t + 1) * 128, :])
                        hc = p8x.tile([128, HID], BF, tag="hc", name="hc")
                        nc.sync.dma_start(
                            hc[:],
                            agout[ch * AGR + OFF_HID + t * 512:
                                  ch * AGR + OFF_HID + (t + 1) * 512, :]
                            .rearrange("(p a) c -> p (a c)", p=128))
                        xb = p8x.tile([128, HID], F16, tag="xc", name="xc", bufs=TSUB)
                        nc.vector.tensor_add(xb[:], oc[:], hc[:])
                        x2c.append(xb)
                        xs = p8x.tile([128, HID], F16, tag="xs", name="xs", bufs=TSUB)
                        nc.scalar.activation(xs[:], xb[:], AF.Copy, scale=0.125)
                        x2s.append(xs)
                    ynT = [p8y.tile([128, TL], BF, tag=f"ynT{k}", name=f"ynT{k}", bufs=1)
                           for k in range(KH)]
                    for t in range(TSUB):
                        sq = p8sq.tile([128, HID], F32, tag="sq", name="sq")
                        nc.vector.tensor_mul(sq[:], x2c[t][:], x2c[t][:])
                        ssum = p8.tile([128, 1], F32, tag="ssum", name="ssum")
                        nc.vector.reduce_sum(out=ssum[:], in_=sq[:], axis=AX.X)
                        rsc = p8.tile([128, 1], F32, tag="rsc", name="rsc")
                        nc.scalar.activation(rsc[:], ssum[:], AF.Sqrt, scale=1.0 / HID, bias=eps_sb[:])
                        nc.vector.reciprocal(rsc[:], rsc[:])
                        yt = p8.tile([128, HID], BF, tag="yn", name="yn", bufs=2)
                        nc.vector.tensor_scalar_mul(yt[:], x2c[t][:], rsc[:])
                        for k in range(KH):
                            ps = p8pst.tile([128, 128], BF, tag="tr", name="tr")
                            nc.tensor.transpose(ps[:], yt[:, k * 128:(k + 1) * 128], ident[:])
                            nc.scalar.copy(ynT[k][:, t * 128:(t + 1) * 128], ps[:])
                    hT = []
                    for i in range(ICL):
                        wgq = p8w.tile([128, KH, 128], I8, tag="wgq", name="wgq")
                        nc.sync.dma_start(wgq[:], wg_e[i])
                        wg_sb = p8w.tile([128, KH, 128], BF, tag="wg", name="wg")
                        nc.scalar.copy(wg_sb[:], wgq[:])
                        wuq = p8w.tile([128, KH, 128], I8, tag="wuq", name="wuq")
                        nc.sync.dma_start(wuq[:], wu_e[i])
                        wu_sb = p8w.tile([128, KH, 128], BF, tag="wu", name="wu")
                        nc.scalar.copy(wu_sb[:], wuq[:])
                        ps_g = p8ps.tile([128, TL], F32, tag="psg", name="psg")
                        ps_u = p8ps.tile([128, TL], F32, tag="psu", name="psu")
                        for k in range(KH):
                            nc.tensor.matmul(ps_g[:], wg_sb[:, k, :], ynT[k][:],
                                             start=(k == 0), stop=(k == KH - 1))
                        for k in range(KH):
                            nc.tensor.matmul(ps_u[:], wu_sb[:, k, :], ynT[k][:],
                                             start=(k == 0), stop=(k == KH - 1))
                        sig = p8.tile([128, TL], BF, tag="sig", name="sig")
                        nc.scalar.activation(sig[:], ps_g[:], AF.Silu,
                                             scale=sg_sb[:, i:i + 1])
                        ub = p8.tile([128, TL], BF, tag="ub", name="ub")
                        nc.vector.tensor_scalar_mul(ub[:], ps_u[:], su_sb[:, i:i + 1])
                        ht = p8h.tile([128, TL], BF, tag="hT", name="hT", bufs=ICL + 2)
                        nc.vector.tensor_mul(ht[:], sig[:], ub[:])
                        hT.append(ht)
                    for tt in range(TSUB):
                        for ng in range(HID // 512):
                            ps_d = p8psd.tile([128, 512], F32, tag="psd", name="psd")
                            for i in range(ICL):
                                nc.tensor.matmul(ps_d[:], hT[i][:, tt * 128:(tt + 1) * 128],
                                                 wd_sb[i][:, ng * 512:(ng + 1) * 512],
                                                 start=(i == 0), stop=(i == ICL - 1))
                            ob = p8.tile([128, 512], F32, tag="ob", name="ob")
                            nc.vector.tensor_add(
                                ob[:], ps_d[:], x2s[tt][:, ng * 512:(ng + 1) * 512])
                            nc.sync.dma_start(
                                rs2_in[ch * TL + tt * 128: ch * TL + (tt + 1) * 128,
                                       ng * 512:(ng + 1) * 512], ob[:])

            # ============ phase 9: ReduceScatter -> finished output ==========
            nc.gpsimd.collective_compute(
                "ReduceScatter", mybir.AluOpType.add,
                replica_groups=[list(range(N))],
                ins=[rs2_in.opt()], outs=[rs2_out.opt()],
            )
            with tc.tile_pool(name="p9", bufs=4) as p9:
                for t in range(TSUB):
                    fin = p9.tile([128, HID], F32, tag="fin", name="fin")
                    nc.sync.dma_start(fin[:], rs2_out[t * 128:(t + 1) * 128, :])
                    fb = p9.tile([128, HID], F16, tag="fb", name="fb")
                    nc.scalar.copy(fb[:], fin[:])
                    nc.sync.dma_start(out_e[t * 128:(t + 1) * 128, :], fb[:])
    return nc


# ---------------------------------------------------------------------------
# Host-side prep
# ---------------------------------------------------------------------------
def _yarn_tables(position_ids, d_rope):
    ar = np.arange(0, d_rope, 2, dtype=np.float32) / d_rope
    freq_extra = 1.0 / BASE ** ar
    freq_inter = 1.0 / (FACTOR * BASE ** ar)

    def corr_dim(num_rot):
        return d_rope * math.log(ORIG_MAX / (num_rot * 2 * math.pi)) / (2 * math.log(BASE))

    low = max(math.floor(corr_dim(BETA_FAST)), 0)
    high = min(math.ceil(corr_dim(BETA_SLOW)), d_rope - 1)
    hi = high + 0.001 if low == high else high
    ramp = np.clip((np.arange(d_rope // 2, dtype=np.float32) - low) / (hi - low), 0.0, 1.0)
    inv_freq_mask = 1.0 - ramp
    inv_freq = freq_inter * (1 - inv_freq_mask) + freq_extra * inv_freq_mask

    def get_mscale(s, m):
        return 1.0 if s <= 1 else 0.1 * m * math.log(s) + 1.0

    ms = get_mscale(FACTOR, MSCALE) / get_mscale(FACTOR, MSCALE_ALL)
    pos = np.asarray(position_ids).reshape(-1).astype(np.float32)
    fr = np.outer(pos, inv_freq)
    return (np.cos(fr) * ms).astype(np.float32), (np.sin(fr) * ms).astype(np.float32)


def _deint_perm(d):
    p = np.empty(d, np.int64)
    p[:d // 2] = 2 * np.arange(d // 2)
    p[d // 2:] = 2 * np.arange(d // 2) + 1
    return p


def prep_inputs(cfg, hidden_states, position_ids, Wq, Wkva, w_kvln, Wkvb, Wo,
                Wg, Wu, Wd, w_ln1, w_ln2):
    c = _derived(cfg)
    N, HPC = c["N_CORES"], c["HPC"]
    HID, KV, DR, DN, DV, DQ = c["HID"], c["KV"], c["D_ROPE"], c["D_NOPE"], c["D_V"], c["DQ"]
    TL, TT, KH = c["T_LOC"], c["T_TOT"], c["KH"]
    ILOC, ICL, IPAD = c["ILOC"], c["ICL"], c["IPAD"]
    bf = ml_dtypes.bfloat16

    hid_flat = np.ascontiguousarray(hidden_states.reshape(TT, HID)).astype(bf)
    perm = _deint_perm(DR)
    scale = np.float32(DQ ** -0.5)

    Wq = Wq * w_ln1[None, :] * scale
    Wqh = Wq.reshape(cfg["H"], DQ, HID)
    Wqh = np.concatenate([Wqh[:, :DN], Wqh[:, DN:][:, perm]], axis=1)
    Wkva = Wkva * w_ln1[None, :]
    Wkva = np.concatenate([Wkva[:KV], Wkva[KV:][perm]], axis=0)
    wkvaT_f = np.ascontiguousarray(Wkva.T)
    skv = np.float32(max(np.abs(wkvaT_f).max() / 127.0, 1e-30))
    wkvaT = np.clip(np.round(wkvaT_f / skv), -127, 127).astype(np.int8)
    Wkvb = Wkvb * w_kvln[None, :]
    Wkvbh = Wkvb.reshape(cfg["H"], DN + DV, KV)
    WoT_f = np.ascontiguousarray(Wo.T, dtype=np.float32)
    WgT_f = (Wg * w_ln2[None, :]).T          # [HID, INTER]
    WuT_f = (Wu * w_ln2[None, :]).T
    WdT_f = Wd.T                             # [INTER, HID]

    def _quant_cols(w):
        # per-column symmetric int8: w[:, i] = q[:, i] * s[i]
        s = np.abs(w).max(axis=0) / 127.0
        s[s == 0] = 1.0
        q = np.clip(np.round(w / s[None, :]), -127, 127).astype(np.int8)
        return q, s.astype(np.float32)

    def _mlp_slices(core):
        i0 = core * ILOC
        gc = np.zeros((HID, IPAD), np.float32)
        gc[:, :ILOC] = WgT_f[:, i0:i0 + ILOC]
        uc = np.zeros((HID, IPAD), np.float32)
        uc[:, :ILOC] = WuT_f[:, i0:i0 + ILOC]
        dc = np.zeros((IPAD, HID), np.float32)
        dc[:ILOC] = WdT_f[i0:i0 + ILOC]
        gq, sg = _quant_cols(gc)
        uq, su = _quant_cols(uc)
        dqT, sd = _quant_cols(dc.T)                          # per-row of dc
        dq = np.ascontiguousarray(dqT.T)
        wg3 = np.ascontiguousarray(gq.reshape(KH, 128, ICL, 128).transpose(2, 1, 0, 3))
        wu3 = np.ascontiguousarray(uq.reshape(KH, 128, ICL, 128).transpose(2, 1, 0, 3))
        wd3 = np.ascontiguousarray(dq.reshape(ICL, 128, HID))
        sg2 = np.ascontiguousarray(sg.reshape(ICL, 128).T)   # [i_inner, i_tile]
        su2 = np.ascontiguousarray(su.reshape(ICL, 128).T)
        sd2 = np.ascontiguousarray(sd.reshape(ICL, 128).T)
        return wg3, wu3, wd3, sg2, su2, sd2

    cos_f, sin_f = _yarn_tables(position_ids, DR)

    in_maps = []
    for core in range(N):
        h0 = core * HPC
        wqT_f = np.ascontiguousarray(
            Wqh[h0:h0 + HPC].transpose(2, 0, 1).reshape(HID, HPC * DQ))
        swq = np.float32(max(np.abs(wqT_f).max() / 127.0, 1e-30))
        wqT = np.clip(np.round(wqT_f / swq), -127, 127).astype(np.int8)
        wbnT = np.ascontiguousarray(
            Wkvbh[h0:h0 + HPC, :DN].transpose(2, 0, 1).reshape(KV, HPC * DN)).astype(bf)
        wbvT = np.ascontiguousarray(
            Wkvbh[h0:h0 + HPC, DN:].transpose(2, 0, 1).reshape(KV, HPC * DV)).astype(bf)
        wg3, wu3, wd3, sg2, su2, sd2 = _mlp_slices(core)
        wo_f = np.ascontiguousarray(WoT_f[h0 * DV:(h0 + HPC) * DV])
        swo = np.float32(max(np.abs(wo_f).max() / 127.0, 1e-30))
        woq8 = np.clip(np.round(wo_f / swo), -127, 127).astype(np.int8)
        sl = slice(core * TL, (core + 1) * TL)
        in_maps.append({
            "hid": hid_flat[sl],
            "wqT": wqT,
            "wkvaT": wkvaT,
            "wbnT": wbnT,
            "wbvT": wbvT,
            "woT": woq8,
            "satt": np.broadcast_to(
                np.array([swq, skv, swo], np.float32), (128, 3)).copy(),
            "wg3": wg3,
            "wu3": wu3,
            "wd3": wd3,
            "sg": sg2,
            "su": su2,
            "sd": sd2,
            "cosL": np.ascontiguousarray(cos_f[sl]),
            "sinL": np.ascontiguousarray(sin_f[sl]),
        })
    return in_maps


def run_cfg(cfg, nc, inputs_dict):
    from concourse.bass_utils import run_bass_kernel_spmd
    c = _derived(cfg)
    in_maps = prep_inputs(cfg, **inputs_dict)
    res = run_bass_kernel_spmd(nc, in_maps, list(range(cfg["N_CORES"])))
    out = np.concatenate(
        [res.results[i]["out"] for i in range(cfg["N_CORES"])], axis=0)
    return out.reshape(cfg["B"], cfg["S"], cfg["HID"]).astype(np.float32), res


_NC_CACHE = {}


def kernel(hidden_states, position_ids, Wq, Wkva, w_kvln, Wkvb, Wo, Wg, Wu, Wd,
           w_ln1, w_ln2):
    cfg = FULL_CFG
    if "full" not in _NC_CACHE:
        _NC_CACHE["full"] = build_kernel(cfg)
    out, _ = run_cfg(cfg, _NC_CACHE["full"], dict(
        hidden_states=np.asarray(hidden_states, np.float32),
        position_ids=np.asarray(position_ids),
        Wq=np.asarray(Wq, np.float32), Wkva=np.asarray(Wkva, np.float32),
        w_kvln=np.asarray(w_kvln, np.float32), Wkvb=np.asarray(Wkvb, np.float32),
        Wo=np.asarray(Wo, np.float32), Wg=np.asarray(Wg, np.float32),
        Wu=np.asarray(Wu, np.float32), Wd=np.asarray(Wd, np.float32),
        w_ln1=np.asarray(w_ln1, np.float32), w_ln2=np.asarray(w_ln2, np.float32)))
    return out



# revision 49
# speedup vs baseline: 1.0785x; 1.0070x over previous
"""DeepseekV2-Lite decoder layer on 8 Trainium2 NeuronCores.

Sharding: attention is tensor-parallel over heads (2 heads/core, all tokens);
o_proj is row-parallel; the MLP is tensor-parallel over the intermediate dim
(1368 cols/core, int8 weights + runtime scales) so gate/up/down weights are
sharded 8x instead of replicated. Three collectives total: AllGather of
(x_norm^T, c_norm^T, k_pe^T, cos^T, sin^T, token-major hid), AllReduce of
o_proj partials (giving every core all-token x2 inputs for the MLP), and a
ReduceScatter of down_proj partials with x2/8 folded in so its output IS the
finished layer output. Matmuls run in bf16 with fp32 PSUM accumulation.
"""
import math
import sys

sys.path.insert(0, "/opt/trn_rl_repo")

import numpy as np
import ml_dtypes

import concourse.bass as bass
import concourse.mybir as mybir
import concourse.tile as tile
from concourse.masks import make_identity

# ---------------------------------------------------------------------------
# Patch: the hardware CTRL instruction supports only one sync-wait slot, but
# kernels with collectives need several on the final Tile drain. Split the
# excess onto SP nops emitted right after the drain, before the sem-clear.
# ---------------------------------------------------------------------------
from concourse.vector_clock import ScopedClock


def _drain_and_barrier_split(self, tick_clock, wait_clock):
    drain_inst = self.nc.sync.drain()
    wait_clock.add_sem_waits(
        drain_inst.ins, ScopedClock({None: tick_clock.global_clock})
    )
    si = drain_inst.ins.sync_info
    if si is not None and len(si.on_wait) > 1:
        waits = list(si.on_wait)
        drain_inst.ins.sync_info = mybir.SyncInfo(
            on_wait=waits[:1], on_update=list(si.on_update)
        )
        for w in waits[1:]:
            nop = self.nc.sync.nop(nofuse=True, hint="drain_wait_overflow")
            nop.ins.sync_info = mybir.SyncInfo(on_wait=[w], on_update=[])
    self.nc.all_engine_barrier()
    assert self.sems is not None
    popped = self.nc._tile_sem_poison_stack.pop()
    assert popped is self._sem_poison
    self.nc.clear_and_free_semaphores(list(self.sems.allocated().values()))
    self.nc.all_engine_barrier()


tile.TileContext._drain_and_barrier = _drain_and_barrier_split

# ---------------------------------------------------------------------------
# Several instruction encodings (DMA, CTRL) accept only one sync-wait slot.
# Split every multi-wait instruction at BIR-serialization time: excess waits
# move onto same-engine NoOps inserted immediately before the instruction.
# ---------------------------------------------------------------------------
import orjson as _orjson

if not getattr(bass.Bass, "_wait_split_patched", False):
    bass.Bass._orig_to_json_bytes = bass.Bass.to_json_bytes
    bass.Bass._wait_split_patched = True
_orig_to_json_bytes = bass.Bass._orig_to_json_bytes


def _to_json_bytes_split(self):
    data = _orjson.loads(_orig_to_json_bytes(self))
    ctr = 0
    for f in data.get("functions", []):
        for bb in f.get("basic_blocks", f.get("blocks", [])):
            insts = bb.get("instructions", [])
            out = []
            for inst in insts:
                si = inst.get("sync_info")
                if si and len(si.get("on_wait") or []) > 1:
                    waits = si["on_wait"]
                    for w in waits[:-1]:
                        ctr += 1
                        out.append({
                            "debug": inst.get("debug", 0),
                            "engine": inst["engine"],
                            "ins": [], "name": f"I-ws{ctr}",
                            "opcode": "NoOp", "outs": [],
                            "sync_info": {"on_update": [], "on_wait": [w]},
                            "text_hint": "wait_split",
                        })
                    si["on_wait"] = [waits[-1]]
                out.append(inst)
            bb["instructions"] = out
    return _orjson.dumps(data)


bass.Bass.to_json_bytes = _to_json_bytes_split

# ---------------------------------------------------------------------------
FULL_CFG = dict(
    B=2, S=2048, HID=2048, H=16, D_NOPE=128, D_ROPE=64, D_V=128, KV=512,
    INTER=10944, N_CORES=8,
)
EPS = 1e-6
MAX_POS, BASE, FACTOR, ORIG_MAX = 8192, 10000.0, 40.0, 4096
BETA_FAST, BETA_SLOW, MSCALE, MSCALE_ALL = 32, 1, 0.707, 0.707

BF = mybir.dt.bfloat16
F32 = mybir.dt.float32
I8 = mybir.dt.int8
F16 = mybir.dt.float16
AX = mybir.AxisListType
AF = mybir.ActivationFunctionType


def _derived(cfg):
    d = dict(cfg)
    d["T_TOT"] = cfg["B"] * cfg["S"]
    d["T_LOC"] = d["T_TOT"] // cfg["N_CORES"]
    d["HPC"] = cfg["H"] // cfg["N_CORES"]
    d["KH"] = cfg["HID"] // 128
    d["KC"] = cfg["KV"] // 128
    d["TSUB"] = d["T_LOC"] // 128
    d["NCH"] = d["T_TOT"] // d["T_LOC"]
    d["ILOC"] = cfg["INTER"] // cfg["N_CORES"]   # 1368 intermediate cols/core
    d["ICL"] = (d["ILOC"] + 127) // 128          # 11 padded k-tiles/core
    d["IPAD"] = d["ICL"] * 128                   # 1408
    d["QTILES_B"] = cfg["S"] // 512
    d["KB_B"] = cfg["S"] // 128
    d["DQ"] = cfg["D_NOPE"] + cfg["D_ROPE"]
    # xnT + cnT + kpeT + cosT + sinT + flat token-major hid
    d["AGROWS"] = 2 * cfg["HID"] + cfg["KV"] + 2 * cfg["D_ROPE"]
    return d


# ---------------------------------------------------------------------------
def build_kernel(cfg):
    c = _derived(cfg)
    N = c["N_CORES"]
    HID, KV, DR, DN, DV = c["HID"], c["KV"], c["D_ROPE"], c["D_NOPE"], c["D_V"]
    TL, TT = c["T_LOC"], c["T_TOT"]
    KH, KC, TSUB, NCH, ICL = c["KH"], c["KC"], c["TSUB"], c["NCH"], c["ICL"]
    HPC, DQ = c["HPC"], c["DQ"]
    QT_B, KB_B = c["QTILES_B"], c["KB_B"]
    B = c["B"]
    HR = DR // 2
    AGR = c["AGROWS"]

    nc = bass.Bass()
    hid_e = nc.dram_tensor("hid", [TL, HID], BF, kind="ExternalInput")
    wqT_e = nc.dram_tensor("wqT", [HID, HPC * DQ], I8, kind="ExternalInput")
    wkvaT_e = nc.dram_tensor("wkvaT", [HID, KV + DR], I8, kind="ExternalInput")
    wbnT_e = nc.dram_tensor("wbnT", [KV, HPC * DN], BF, kind="ExternalInput")
    wbvT_e = nc.dram_tensor("wbvT", [KV, HPC * DV], BF, kind="ExternalInput")
    woT_e = nc.dram_tensor("woT", [HPC * DV, HID], I8, kind="ExternalInput")
    wg_e = nc.dram_tensor("wg3", [ICL, 128, KH, 128], I8, kind="ExternalInput")
    wu_e = nc.dram_tensor("wu3", [ICL, 128, KH, 128], I8, kind="ExternalInput")
    wd_e = nc.dram_tensor("wd3", [ICL, 128, HID], I8, kind="ExternalInput")
    satt_e = nc.dram_tensor("satt", [128, 3], F32, kind="ExternalInput")
    sg_e = nc.dram_tensor("sg", [128, ICL], F32, kind="ExternalInput")
    su_e = nc.dram_tensor("su", [128, ICL], F32, kind="ExternalInput")
    sd_e = nc.dram_tensor("sd", [128, ICL], F32, kind="ExternalInput")
    cosL_e = nc.dram_tensor("cosL", [TL, HR], F32, kind="ExternalInput")
    sinL_e = nc.dram_tensor("sinL", [TL, HR], F32, kind="ExternalInput")
    out_e = nc.dram_tensor("out", [TL, HID], F16, kind="ExternalOutput")
    probe = cfg.get("probe", False)
    if probe:
        p_agin_e = nc.dram_tensor("p_agin", [AGR, TL], BF, kind="ExternalOutput")

    with tile.TileContext(nc) as tc:
        with (
            tc.tile_pool(name="dram", bufs=1, space="DRAM") as dram,
            tc.tile_pool(name="const", bufs=1) as const,
        ):
            agin = dram.tile([AGR, TL], BF, tag="agin", name="agin")
            agout = dram.tile([N * AGR, TL], BF, addr_space="Shared", tag="agout", name="agout")
            rs_in = dram.tile([TT, HID], F16, tag="rsin", name="rsin")
            x2a = dram.tile([TT, HID], F16, addr_space="Shared", tag="x2a", name="x2a")
            rs2_in = dram.tile([TT, HID], F16, tag="rs2in", name="rs2in")
            rs2_out = dram.tile([TL, HID], F16, tag="rs2out", name="rs2out")
            OFF_KPE = HID + KV
            OFF_COS = OFF_KPE + DR
            OFF_SIN = OFF_COS + HR
            OFF_HID = OFF_SIN + HR

            ident = const.tile([128, 128], BF, tag="ident", name="ident")
            make_identity(nc, ident)
            eps_sb = const.tile([128, 1], F32, tag="eps", name="eps")
            nc.vector.memset(eps_sb[:], EPS)
            # mask[p, x] = 1.0 where x >= p + 384, else 0 — generated on device
            mask_sb = const.tile([128, 896], BF, tag="mask", name="mask")
            nc.gpsimd.memset(mask_sb[:], 1.0)
            nc.gpsimd.affine_select(
                out=mask_sb[:], in_=mask_sb[:],
                compare_op=mybir.AluOpType.is_ge, fill=0.0,
                base=-384, pattern=[[1, 896]], channel_multiplier=-1)
            cosL_sb = const.tile([128, TSUB, HR], F32, tag="cosL", name="cosL")
            nc.sync.dma_start(cosL_sb[:], cosL_e.rearrange("(a p) r -> p a r", p=128))
            sinL_sb = const.tile([128, TSUB, HR], F32, tag="sinL", name="sinL")
            nc.sync.dma_start(sinL_sb[:], sinL_e.rearrange("(a p) r -> p a r", p=128))
            satt_sb = const.tile([128, 3], F32, tag="satt", name="satt")
            nc.sync.dma_start(satt_sb[:], satt_e[:])
            sg_sb = const.tile([128, ICL], F32, tag="sg", name="sg")
            nc.sync.dma_start(sg_sb[:], sg_e[:])
            su_sb = const.tile([128, ICL], F32, tag="su", name="su")
            nc.sync.dma_start(su_sb[:], su_e[:])
            sd_sb = const.tile([128, ICL], F32, tag="sd", name="sd")
            nc.sync.dma_start(sd_sb[:], sd_e[:])

            # ============ phases 0-1: rms1, x^T, ckv, rms(c), rope(k_pe) =====
            with (
                tc.tile_pool(name="xnTp", bufs=1) as xnTp,
                tc.tile_pool(name="p0", bufs=2) as p0,
                tc.tile_pool(name="p01ps", bufs=2, space="PSUM") as p01ps,
            ):
                xnT = [xnTp.tile([128, TL], BF, tag=f"xnT{k}", name=f"xnT{k}") for k in range(KH)]
                # token-major hid rides the AllGather as a flat [TL*HID/TL, TL] region
                nc.sync.dma_start(
                    agin[OFF_HID:OFF_HID + HID, :],
                    hid_e.rearrange("t (a c) -> (t a) c", c=TL))
                xn_sb = []
                for t in range(TSUB):
                    ht = p0.tile([128, HID], BF, tag="hid0", name="hid0")
                    nc.sync.dma_start(ht[:], hid_e[t * 128:(t + 1) * 128, :])
                    sq = p0.tile([128, HID], F32, tag="sq", name="sq")
                    nc.vector.tensor_mul(sq[:], ht[:], ht[:])
                    ssum = p0.tile([128, 1], F32, tag="ssum", name="ssum")
                    nc.vector.reduce_sum(out=ssum[:], in_=sq[:], axis=AX.X)
                    rs = p0.tile([128, 1], F32, tag="rs", name="rs")
                    nc.scalar.activation(rs[:], ssum[:], AF.Sqrt, scale=1.0 / HID, bias=eps_sb[:])
                    nc.vector.reciprocal(rs[:], rs[:])
                    xt = p0.tile([128, HID], BF, tag="xn", name="xn", bufs=TSUB)
                    nc.vector.tensor_scalar_mul(xt[:], ht[:], rs[:])
                    xn_sb.append(xt)
                for t in range(TSUB):
                    for k in range(KH):
                        ps = p01ps.tile([128, 128], BF, tag="tr", name="tr")
                        nc.tensor.transpose(ps[:], xn_sb[t][:, k * 128:(k + 1) * 128], ident[:])
                        nc.scalar.copy(xnT[k][:, t * 128:(t + 1) * 128], ps[:])
                for k in range(KH):
                    nc.sync.dma_start(agin[k * 128:(k + 1) * 128, :], xnT[k][:])

                # phase 1
                wkva_sb = [p0.tile([128, KV + DR], BF, tag=f"wkva{k}", name=f"wkva{k}") for k in range(KH)]
                for k in range(KH):
                    wkq = p0.tile([128, KV + DR], I8, tag="wkq", name="wkq")
                    nc.sync.dma_start(wkq[:], wkvaT_e[k * 128:(k + 1) * 128, :])
                    nc.scalar.copy(wkva_sb[k][:], wkq[:])
                # cos/sin scaled by s_kv for the k_pe rope (k_pe psum is raw int units)
                clk = p0.tile([128, TSUB, HR], F32, tag="clk", name="clk", bufs=1)
                nc.vector.tensor_scalar_mul(clk[:], cosL_sb[:], satt_sb[:, 1:2])
                slk = p0.tile([128, TSUB, HR], F32, tag="slk", name="slk", bufs=1)
                nc.vector.tensor_scalar_mul(slk[:], sinL_sb[:], satt_sb[:, 1:2])
                cnT_sb = [p0.tile([128, TL], BF, tag=f"cnT{j}", name=f"cnT{j}") for j in range(KC)]
                kpeT_loc = p0.tile([DR, TL], BF, tag="kpeT_loc", name="kpeT_loc")
                for t in range(TSUB):
                    ps_c = p01ps.tile([128, KV], F32, tag="psc", name="psc")
                    ps_p = p01ps.tile([128, DR], F32, tag="psp", name="psp")
                    for k in range(KH):
                        lq = xnT[k][:, t * 128:(t + 1) * 128]
                        nc.tensor.matmul(ps_c[:], lq, wkva_sb[k][:, :KV],
                                         start=(k == 0), stop=(k == KH - 1))
                        nc.tensor.matmul(ps_p[:], lq, wkva_sb[k][:, KV:],
                                         start=(k == 0), stop=(k == KH - 1))
                    sq = p0.tile([128, KV], F32, tag="sqc", name="sqc")
                    nc.scalar.activation(sq[:], ps_c[:], AF.Square)
                    ssum = p0.tile([128, 1], F32, tag="ssumc", name="ssumc")
                    nc.vector.reduce_sum(out=ssum[:], in_=sq[:], axis=AX.X)
                    rs = p0.tile([128, 1], F32, tag="rsc", name="rsc")
                    nc.scalar.activation(rs[:], ssum[:], AF.Sqrt, scale=1.0 / KV, bias=eps_sb[:])
                    nc.vector.reciprocal(rs[:], rs[:])
                    cn = p0.tile([128, KV], BF, tag="cn", name="cn")
                    nc.vector.tensor_scalar_mul(cn[:], ps_c[:], rs[:])
                    kp = p0.tile([128, DR], BF, tag="kp", name="kp")
                    a = p0.tile([128, HR], F32, tag="ra", name="ra")
                    b = p0.tile([128, HR], F32, tag="rb", name="rb")
                    cosl = clk[:, t, :]
                    sinl = slk[:, t, :]
                    nc.vector.tensor_mul(a[:], ps_p[:, :HR], cosl)
                    nc.vector.tensor_mul(b[:], ps_p[:, HR:], sinl)
                    nc.vector.tensor_sub(kp[:, :HR], a[:], b[:])
                    nc.vector.tensor_mul(a[:], ps_p[:, HR:], cosl)
                    nc.vector.tensor_mul(b[:], ps_p[:, :HR], sinl)
                    nc.vector.tensor_add(kp[:, HR:], a[:], b[:])
                    for j in range(KC):
                        ps = p01ps.tile([128, 128], BF, tag="tr", name="tr")
                        nc.tensor.transpose(ps[:], cn[:, j * 128:(j + 1) * 128], ident[:])
                        nc.scalar.copy(cnT_sb[j][:, t * 128:(t + 1) * 128], ps[:])
                    ps = p01ps.tile([128, 128], BF, tag="tr", name="tr")
                    nc.tensor.transpose(ps[:DR, :], kp[:], ident[:])
                    nc.scalar.copy(kpeT_loc[:, t * 128:(t + 1) * 128], ps[:DR, :])
                for j in range(KC):
                    nc.sync.dma_start(agin[HID + j * 128:HID + (j + 1) * 128, :], cnT_sb[j][:])
                nc.sync.dma_start(agin[OFF_KPE:OFF_KPE + DR, :], kpeT_loc[:])
                # ride local cos/sin (transposed, bf16) for the q-rope phase
                cl_bf = p0.tile([128, TSUB, HR], BF, tag="clbf", name="clbf", bufs=1)
                nc.vector.tensor_scalar_mul(cl_bf[:], cosL_sb[:], satt_sb[:, 0:1])
                sl_bf = p0.tile([128, TSUB, HR], BF, tag="slbf", name="slbf", bufs=1)
                nc.vector.tensor_scalar_mul(sl_bf[:], sinL_sb[:], satt_sb[:, 0:1])
                cosT_loc = p0.tile([HR, TL], BF, tag="cosTl", name="cosTl", bufs=1)
                sinT_loc = p0.tile([HR, TL], BF, tag="sinTl", name="sinTl", bufs=1)
                for t in range(TSUB):
                    ps = p01ps.tile([128, 128], BF, tag="tr", name="tr")
                    nc.tensor.transpose(ps[:HR, :], cl_bf[:, t, :], ident[:])
                    nc.scalar.copy(cosT_loc[:, t * 128:(t + 1) * 128], ps[:HR, :])
                    ps = p01ps.tile([128, 128], BF, tag="tr", name="tr")
                    nc.tensor.transpose(ps[:HR, :], sl_bf[:, t, :], ident[:])
                    nc.scalar.copy(sinT_loc[:, t * 128:(t + 1) * 128], ps[:HR, :])
                nc.sync.dma_start(agin[OFF_COS:OFF_COS + HR, :], cosT_loc[:])
                nc.sync.dma_start(agin[OFF_SIN:OFF_SIN + HR, :], sinT_loc[:])

            # ============ phase 2: AllGather ================================
            nc.gpsimd.collective_compute(
                "AllGather", mybir.AluOpType.bypass,
                replica_groups=[list(range(N))],
                ins=[agin.opt()], outs=[agout.opt()],
            )

            if probe:
                with tc.tile_pool(name="prb0", bufs=2) as prb0:
                    for r in range(0, AGR, 128):
                        w = min(128, AGR - r)
                        pt_ = prb0.tile([128, TL], BF, tag="pgt", name="pgt")
                        nc.sync.dma_start(pt_[:w, :], agin[r:r + w, :])
                        nc.sync.dma_start(p_agin_e[r:r + w, :], pt_[:w, :])

            # ============ phases 3-5: attention ==============================
            with tc.tile_pool(name="asb", bufs=1) as asb:
                qnT = [asb.tile([128, TT], BF, tag=f"qnT{h}", name=f"qnT{h}") for h in range(HPC)]
                qpT = [asb.tile([DR, TT], BF, tag=f"qpT{h}", name=f"qpT{h}") for h in range(HPC)]
                knT = [asb.tile([128, TT], BF, tag=f"knT{h}", name=f"knT{h}") for h in range(HPC)]
                kpeT = asb.tile([DR, TT], BF, tag="kpeT", name="kpeT")
                v_sb = [asb.tile([128, TT // 128, DV + 4], BF, tag=f"v{h}", name=f"v{h}")
                        for h in range(HPC)]
                atT = [asb.tile([128, TT], BF, tag=f"atT{h}", name=f"atT{h}") for h in range(HPC)]
                cosT_sb = asb.tile([HR, TT], BF, tag="cosT", name="cosT")
                sinT_sb = asb.tile([HR, TT], BF, tag="sinT", name="sinT")

                with (
                    tc.tile_pool(name="p4w", bufs=1) as p4w,
                    tc.tile_pool(name="p4x", bufs=1) as p4x,
                    tc.tile_pool(name="p4", bufs=2) as p4,
                    tc.tile_pool(name="p4ps", bufs=2, space="PSUM") as p4ps,
                ):
                    wq_sb = [p4w.tile([128, HPC * DQ], BF, tag=f"wq{k}", name=f"wq{k}") for k in range(KH)]
                    for k in range(KH):
                        wqq = p4.tile([128, HPC * DQ], I8, tag="wqq", name="wqq")
                        nc.sync.dma_start(wqq[:], wqT_e[k * 128:(k + 1) * 128, :])
                        nc.scalar.copy(wq_sb[k][:], wqq[:])
                    wbn_sb = [p4w.tile([128, HPC * DN], BF, tag=f"wbn{j}", name=f"wbn{j}") for j in range(KC)]
                    wbv_sb = [p4w.tile([128, HPC * DV], BF, tag=f"wbv{j}", name=f"wbv{j}") for j in range(KC)]
                    for j in range(KC):
                        nc.sync.dma_start(wbn_sb[j][:], wbnT_e[j * 128:(j + 1) * 128, :])
                        nc.sync.dma_start(wbv_sb[j][:], wbvT_e[j * 128:(j + 1) * 128, :])

                    for ch in range(NCH):
                        nc.sync.dma_start(
                            kpeT[:, ch * TL:(ch + 1) * TL],
                            agout[ch * AGR + OFF_KPE: ch * AGR + OFF_KPE + DR, :])
                        nc.sync.dma_start(
                            cosT_sb[:, ch * TL:(ch + 1) * TL],
                            agout[ch * AGR + OFF_COS: ch * AGR + OFF_COS + HR, :])
                        nc.sync.dma_start(
                            sinT_sb[:, ch * TL:(ch + 1) * TL],
                            agout[ch * AGR + OFF_SIN: ch * AGR + OFF_SIN + HR, :])

                    for ch in range(NCH):
                        xch = []
                        for k in range(KH):
                            xt = p4x.tile([128, TL], BF, tag="xch", name="xch", bufs=KH + 4)
                            nc.sync.dma_start(
                                xt[:], agout[ch * AGR + k * 128: ch * AGR + (k + 1) * 128, :])
                            xch.append(xt)
                        cs = slice(ch * TL, (ch + 1) * TL)
                        for h in range(HPC):
                            ps_n = p4ps.tile([128, TL], F32, tag="qn", name="qn")
                            ps_p = p4ps.tile([DR, TL], F32, tag="qp", name="qp")
                            off = h * DQ
                            for k in range(KH):
                                nc.tensor.matmul(ps_n[:], wq_sb[k][:, off:off + DN], xch[k][:],
                                                 start=(k == 0), stop=(k == KH - 1))
                            for k in range(KH):
                                nc.tensor.matmul(ps_p[:], wq_sb[k][:, off + DN:off + DQ], xch[k][:],
                                                 start=(k == 0), stop=(k == KH - 1))
                            nc.vector.tensor_scalar_mul(qnT[h][:, cs], ps_n[:], satt_sb[:, 0:1])
                            a = p4.tile([HR, TL], F32, tag="qa", name="qa")
                            b = p4.tile([HR, TL], F32, tag="qb", name="qb")
                            cosc = cosT_sb[:, cs]
                            sinc = sinT_sb[:, cs]
                            nc.vector.tensor_mul(a[:], ps_p[:HR, :], cosc)
                            nc.vector.tensor_mul(b[:], ps_p[HR:, :], sinc)
                            nc.vector.tensor_sub(qpT[h][:HR, cs], a[:], b[:])
                            nc.vector.tensor_mul(a[:], ps_p[HR:, :], cosc)
                            nc.vector.tensor_mul(b[:], ps_p[:HR, :], sinc)
                            nc.vector.tensor_add(qpT[h][HR:, cs], a[:], b[:])

                    for ch in range(NCH):
                        cch = []
                        for j in range(KC):
                            ct = p4x.tile([128, TL], BF, tag="cch", name="cch", bufs=KC + 2)
                            nc.sync.dma_start(
                                ct[:], agout[ch * AGR + HID + j * 128: ch * AGR + HID + (j + 1) * 128, :])
                            cch.append(ct)
                        cs = slice(ch * TL, (ch + 1) * TL)
                        for h in range(HPC):
                            ps_k = p4ps.tile([128, TL], F32, tag="kn", name="kn")
                            for j in range(KC):
                                nc.tensor.matmul(ps_k[:], wbn_sb[j][:, h * DN:(h + 1) * DN], cch[j][:],
                                                 start=(j == 0), stop=(j == KC - 1))
                            nc.scalar.copy(knT[h][:, cs], ps_k[:])
                            for j4 in range(TL // 128):
                                ps_v = p4ps.tile([128, DV], F32, tag="pv", name="pv")
                                for j in range(KC):
                                    nc.tensor.matmul(ps_v[:], cch[j][:, j4 * 128:(j4 + 1) * 128],
                                                     wbv_sb[j][:, h * DV:(h + 1) * DV],
                                                     start=(j == 0), stop=(j == KC - 1))
                                kbt = ch * (TL // 128) + j4
                                nc.scalar.copy(v_sb[h][:, kbt, :DV], ps_v[:])
                                nc.vector.memset(v_sb[h][:, kbt, DV:DV + 1], 1.0)

                # ---------------- phase 5: attention -------------------------
                with (
                    tc.tile_pool(name="p5ps", bufs=2, space="PSUM") as p5ps,
                    tc.tile_pool(name="p5pv", bufs=2, space="PSUM") as p5pv,
                    tc.tile_pool(name="p5", bufs=2) as p5,
                    tc.tile_pool(name="prb", bufs=1) as prb,
                ):
                    for b in range(B):
                        for h in range(HPC):
                            for qt in range(QT_B):
                                qs = slice(b * cfg["S"] + qt * 512, b * cfg["S"] + qt * 512 + 512)
                                nkb = 4 * qt + 4
                                pt = []
                                for kb in range(nkb):
                                    kbg = b * KB_B + kb
                                    ks = slice(kbg * 128, kbg * 128 + 128)
                                    ps_s = p5ps.tile([128, 512], F32, tag="ps_s", name="ps_s")
                                    nc.tensor.matmul(ps_s[:], knT[h][:, ks], qnT[h][:, qs],
                                                     start=True, stop=False)
                                    nc.tensor.matmul(ps_s[:], kpeT[:, ks], qpT[h][:, qs],
                                                     start=False, stop=True)
                                    pb = prb.tile([128, 512], BF, tag="pb", name="pb", bufs=KB_B + 4)
                                    nc.scalar.activation(pb[:], ps_s[:], AF.Exp)
                                    delta = kb * 128 - qt * 512
                                    if delta >= 0:
                                        nc.vector.tensor_mul(
                                            pb[:], pb[:], mask_sb[:, 384 - delta:896 - delta])
                                    pt.append(pb)
                                for q4 in range(4):
                                    ps_av = p5pv.tile([128, DV + 4], F32, tag="ps_av", name="ps_av")
                                    for kb in range(nkb):
                                        kbt = b * KB_B + kb
                                        nc.tensor.matmul(
                                            ps_av[:, :DV + 1],
                                            pt[kb][:, q4 * 128:(q4 + 1) * 128],
                                            v_sb[h][:, kbt, :DV + 1],
                                            start=(kb == 0), stop=(kb == nkb - 1))
                                    recip = p5.tile([128, 1], F32, tag="recip", name="recip")
                                    nc.vector.reciprocal(recip[:], ps_av[:, DV:DV + 1])
                                    at = p5.tile([128, DV], BF, tag="at", name="at")
                                    nc.vector.tensor_scalar_mul(at[:], ps_av[:, :DV], recip[:])
                                    ps_t = p5ps.tile([128, 128], BF, tag="ps_t", name="ps_t")
                                    nc.tensor.transpose(ps_t[:DV, :], at[:], ident[:])
                                    qg = (b * cfg["S"] + qt * 512) // 128 + q4
                                    nc.scalar.copy(atT[h][:DV, qg * 128:(qg + 1) * 128], ps_t[:DV, :])

                # ============ phase 5b: row-parallel o_proj partials =============
                with (
                    tc.tile_pool(name="p6w", bufs=1) as p6w,
                    tc.tile_pool(name="p6", bufs=4) as p6,
                    tc.tile_pool(name="p6ps", bufs=4, space="PSUM") as p6ps,
                ):
                    wo_sb = [p6w.tile([128, HID], BF, tag=f"wo{j}", name=f"wo{j}") for j in range(HPC)]
                    for j in range(HPC):
                        woq = p6.tile([128, HID], I8, tag="woq", name="woq")
                        nc.sync.dma_start(woq[:DV, :], woT_e[j * DV:(j + 1) * DV, :])
                        nc.scalar.copy(wo_sb[j][:DV, :], woq[:DV, :])
                    for tq in range(TT // 128):
                        for nsl in range(HID // 512):
                            ps_o = p6ps.tile([128, 512], F32, tag="ps_o", name="ps_o")
                            for j in range(HPC):
                                nc.tensor.matmul(ps_o[:], atT[j][:DV, tq * 128:(tq + 1) * 128],
                                                 wo_sb[j][:, nsl * 512:(nsl + 1) * 512],
                                                 start=(j == 0), stop=(j == HPC - 1))
                            ob = p6.tile([128, 512], F16, tag="ob", name="ob")
                            nc.vector.tensor_scalar_mul(ob[:], ps_o[:], satt_sb[:, 2:3])
                            nc.sync.dma_start(
                                rs_in[tq * 128:(tq + 1) * 128, nsl * 512:(nsl + 1) * 512], ob[:])

            # ============ phase 6: AllReduce o_proj partials ================
            nc.gpsimd.collective_compute(
                "AllReduce", mybir.AluOpType.add,
                replica_groups=[list(range(N))],
                ins=[rs_in.opt()], outs=[x2a.opt()],
            )

            # ============ phases 7-8: x2, rms2, TP MLP over INTER ============
            # Every core: for each 512-token chunk, assemble x2 = o_attn + hid
            # (both all-token), rms2 + transpose to y^T, gate/up/down on its
            # 1368-col INTER slice, fold x2/8 into the down partials so the
            # final ReduceScatter(add) emits the finished layer output.
            with (
                tc.tile_pool(name="p8wd", bufs=1) as p8wd,
                tc.tile_pool(name="p8w", bufs=2) as p8w,
                tc.tile_pool(name="p8x", bufs=2) as p8x,
                tc.tile_pool(name="p8sq", bufs=2) as p8sq,
                tc.tile_pool(name="p8y", bufs=1) as p8y,
                tc.tile_pool(name="p8h", bufs=1) as p8h,
                tc.tile_pool(name="p8", bufs=4) as p8,
                tc.tile_pool(name="p8ps", bufs=2, space="PSUM") as p8ps,
                tc.tile_pool(name="p8psd", bufs=2, space="PSUM") as p8psd,
                tc.tile_pool(name="p8pst", bufs=2, space="PSUM") as p8pst,
            ):
                wd_sb = [p8wd.tile([128, HID], BF, tag=f"wd{i}", name=f"wd{i}")
                         for i in range(ICL)]
                for i in range(ICL):
                    wdq = p8w.tile([128, HID], I8, tag="wdq", name="wdq")
                    nc.sync.dma_start(wdq[:], wd_e[i])
                    nc.vector.tensor_scalar_mul(wd_sb[i][:], wdq[:], sd_sb[:, i:i + 1])
                for ch in range(NCH):
                    x2c, x2s = [], []
                    for t in range(TSUB):
                        oc = p8x.tile([128, HID], F16, tag="oc", name="oc")
                        nc.sync.dma_start(
                            oc[:], x2a[ch * TL + t * 128: ch * TL + (t + 1) * 128, :])
                        hc = p8x.tile([128, HID], BF, tag="hc", name="hc")
                        nc.sync.dma_start(
                            hc[:],
                            agout[ch * AGR + OFF_HID + t * 512:
                                  ch * AGR + OFF_HID + (t + 1) * 512, :]
                            .rearrange("(p a) c -> p (a c)", p=128))
                        xb = p8x.tile([128, HID], F16, tag="xc", name="xc", bufs=TSUB)
                        nc.vector.tensor_add(xb[:], oc[:], hc[:])
                        x2c.append(xb)
                        xs = p8x.tile([128, HID], F16, tag="xs", name="xs", bufs=TSUB)
                        nc.scalar.activation(xs[:], xb[:], AF.Copy, scale=0.125)
                        x2s.append(xs)
                    ynT = [p8y.tile([128, TL], BF, tag=f"ynT{k}", name=f"ynT{k}", bufs=1)
                           for k in range(KH)]
                    for t in range(TSUB):
                        sq = p8sq.tile([128, HID], F32, tag="sq", name="sq")
                        nc.vector.tensor_mul(sq[:], x2c[t][:], x2c[t][:])
                        ssum = p8.tile([128, 1], F32, tag="ssum", name="ssum")
                        nc.vector.reduce_sum(out=ssum[:], in_=sq[:], axis=AX.X)
                        rsc = p8.tile([128, 1], F32, tag="rsc", name="rsc")
                        nc.scalar.activation(rsc[:], ssum[:], AF.Sqrt, scale=1.0 / HID, bias=eps_sb[:])
                        nc.vector.reciprocal(rsc[:], rsc[:])
                        yt = p8.tile([128, HID], BF, tag="yn", name="yn", bufs=2)
                        nc.vector.tensor_scalar_mul(yt[:], x2c[t][:], rsc[:])
                        for k in range(KH):
                            ps = p8pst.tile([128, 128], BF, tag="tr", name="tr")
                            nc.tensor.transpose(ps[:], yt[:, k * 128:(k + 1) * 128], ident[:])
                            nc.scalar.copy(ynT[k][:, t * 128:(t + 1) * 128], ps[:])
                    hT = []
                    for i in range(ICL):
                        wgq = p8w.tile([128, KH, 128], I8, tag="wgq", name="wgq")
                        nc.sync.dma_start(wgq[:], wg_e[i])
                        wg_sb = p8w.tile([128, KH, 128], BF, tag="wg", name="wg")
                        nc.scalar.copy(wg_sb[:], wgq[:])
                        wuq = p8w.tile([128, KH, 128], I8, tag="wuq", name="wuq")
                        nc.sync.dma_start(wuq[:], wu_e[i])
                        wu_sb = p8w.tile([128, KH, 128], BF, tag="wu", name="wu")
                        nc.scalar.copy(wu_sb[:], wuq[:])
                        ps_g = p8ps.tile([128, TL], F32, tag="psg", name="psg")
                        ps_u = p8ps.tile([128, TL], F32, tag="psu", name="psu")
                        for k in range(KH):
                            nc.tensor.matmul(ps_g[:], wg_sb[:, k, :], ynT[k][:],
                                             start=(k == 0), stop=(k == KH - 1))
                        for k in range(KH):
                            nc.tensor.matmul(ps_u[:], wu_sb[:, k, :], ynT[k][:],
                                             start=(k == 0), stop=(k == KH - 1))
                        sig = p8.tile([128, TL], BF, tag="sig", name="sig")
                        nc.scalar.activation(sig[:], ps_g[:], AF.Silu,
                                             scale=sg_sb[:, i:i + 1])
                        ub = p8.tile([128, TL], BF, tag="ub", name="ub")
                        nc.vector.tensor_scalar_mul(ub[:], ps_u[:], su_sb[:, i:i + 1])
                        ht = p8h.tile([128, TL], BF, tag="hT", name="hT", bufs=ICL + 2)
                        nc.vector.tensor_mul(ht[:], sig[:], ub[:])
                        hT.append(ht)
                    for tt in range(TSUB):
                        for ng in range(HID // 512):
                            ps_d = p8psd.tile([128, 512], F32, tag="psd", name="psd")
                            for i in range(ICL):
                                nc.tensor.matmul(ps_d[:], hT[i][:, tt * 128:(tt + 1) * 128],
                                                 wd_sb[i][:, ng * 512:(ng + 1) * 512],
                                                 start=(i == 0), stop=(i == ICL - 1))
                            ob = p8.tile([128, 512], F16, tag="ob", name="ob")
                            nc.vector.tensor_add(
                                ob[:], ps_d[:], x2s[tt][:, ng * 512:(ng + 1) * 512])
                            nc.sync.dma_start(
                                rs2_in[ch * TL + tt * 128: ch * TL + (tt + 1) * 128,
                                       ng * 512:(ng + 1) * 512], ob[:])

            # ============ phase 9: ReduceScatter -> finished output ==========
            nc.gpsimd.collective_compute(
                "ReduceScatter", mybir.AluOpType.add,
                replica_groups=[list(range(N))],
                ins=[rs2_in.opt()], outs=[rs2_out.opt()],
            )
            nc.sync.dma_start(out_e[:, :], rs2_out[:])
    return nc


# ---------------------------------------------------------------------------
# Host-side prep
# ---------------------------------------------------------------------------
def _yarn_tables(position_ids, d_rope):
    ar = np.arange(0, d_rope, 2, dtype=np.float32) / d_rope
    freq_extra = 1.0 / BASE ** ar
    freq_inter = 1.0 / (FACTOR * BASE ** ar)

    def corr_dim(num_rot):
        return d_rope * math.log(ORIG_MAX / (num_rot * 2 * math.pi)) / (2 * math.log(BASE))

    low = max(math.floor(corr_dim(BETA_FAST)), 0)
    high = min(math.ceil(corr_dim(BETA_SLOW)), d_rope - 1)
    hi = high + 0.001 if low == high else high
    ramp = np.clip((np.arange(d_rope // 2, dtype=np.float32) - low) / (hi - low), 0.0, 1.0)
    inv_freq_mask = 1.0 - ramp
    inv_freq = freq_inter * (1 - inv_freq_mask) + freq_extra * inv_freq_mask

    def get_mscale(s, m):
        return 1.0 if s <= 1 else 0.1 * m * math.log(s) + 1.0

    ms = get_mscale(FACTOR, MSCALE) / get_mscale(FACTOR, MSCALE_ALL)
    pos = np.asarray(position_ids).reshape(-1).astype(np.float32)
    fr = np.outer(pos, inv_freq)
    return (np.cos(fr) * ms).astype(np.float32), (np.sin(fr) * ms).astype(np.float32)


def _deint_perm(d):
    p = np.empty(d, np.int64)
    p[:d // 2] = 2 * np.arange(d // 2)
    p[d // 2:] = 2 * np.arange(d // 2) + 1
    return p


def prep_inputs(cfg, hidden_states, position_ids, Wq, Wkva, w_kvln, Wkvb, Wo,
                Wg, Wu, Wd, w_ln1, w_ln2):
    c = _derived(cfg)
    N, HPC = c["N_CORES"], c["HPC"]
    HID, KV, DR, DN, DV, DQ = c["HID"], c["KV"], c["D_ROPE"], c["D_NOPE"], c["D_V"], c["DQ"]
    TL, TT, KH = c["T_LOC"], c["T_TOT"], c["KH"]
    ILOC, ICL, IPAD = c["ILOC"], c["ICL"], c["IPAD"]
    bf = ml_dtypes.bfloat16

    hid_flat = np.ascontiguousarray(hidden_states.reshape(TT, HID)).astype(bf)
    perm = _deint_perm(DR)
    scale = np.float32(DQ ** -0.5)

    Wq = Wq * w_ln1[None, :] * scale
    Wqh = Wq.reshape(cfg["H"], DQ, HID)
    Wqh = np.concatenate([Wqh[:, :DN], Wqh[:, DN:][:, perm]], axis=1)
    Wkva = Wkva * w_ln1[None, :]
    Wkva = np.concatenate([Wkva[:KV], Wkva[KV:][perm]], axis=0)
    wkvaT_f = np.ascontiguousarray(Wkva.T)
    skv = np.float32(max(np.abs(wkvaT_f).max() / 127.0, 1e-30))
    wkvaT = np.clip(np.round(wkvaT_f / skv), -127, 127).astype(np.int8)
    Wkvb = Wkvb * w_kvln[None, :]
    Wkvbh = Wkvb.reshape(cfg["H"], DN + DV, KV)
    WoT_f = np.ascontiguousarray(Wo.T, dtype=np.float32)
    WgT_f = (Wg * w_ln2[None, :]).T          # [HID, INTER]
    WuT_f = (Wu * w_ln2[None, :]).T
    WdT_f = Wd.T                             # [INTER, HID]

    def _quant_cols(w):
        # per-column symmetric int8: w[:, i] = q[:, i] * s[i]
        s = np.abs(w).max(axis=0) / 127.0
        s[s == 0] = 1.0
        q = np.clip(np.round(w / s[None, :]), -127, 127).astype(np.int8)
        return q, s.astype(np.float32)

    def _mlp_slices(core):
        i0 = core * ILOC
        gc = np.zeros((HID, IPAD), np.float32)
        gc[:, :ILOC] = WgT_f[:, i0:i0 + ILOC]
        uc = np.zeros((HID, IPAD), np.float32)
        uc[:, :ILOC] = WuT_f[:, i0:i0 + ILOC]
        dc = np.zeros((IPAD, HID), np.float32)
        dc[:ILOC] = WdT_f[i0:i0 + ILOC]
        gq, sg = _quant_cols(gc)
        uq, su = _quant_cols(uc)
        dqT, sd = _quant_cols(dc.T)                          # per-row of dc
        dq = np.ascontiguousarray(dqT.T)
        wg3 = np.ascontiguousarray(gq.reshape(KH, 128, ICL, 128).transpose(2, 1, 0, 3))
        wu3 = np.ascontiguousarray(uq.reshape(KH, 128, ICL, 128).transpose(2, 1, 0, 3))
        wd3 = np.ascontiguousarray(dq.reshape(ICL, 128, HID))
        sg2 = np.ascontiguousarray(sg.reshape(ICL, 128).T)   # [i_inner, i_tile]
        su2 = np.ascontiguousarray(su.reshape(ICL, 128).T)
        sd2 = np.ascontiguousarray(sd.reshape(ICL, 128).T)
        return wg3, wu3, wd3, sg2, su2, sd2

    cos_f, sin_f = _yarn_tables(position_ids, DR)

    in_maps = []
    for core in range(N):
        h0 = core * HPC
        wqT_f = np.ascontiguousarray(
            Wqh[h0:h0 + HPC].transpose(2, 0, 1).reshape(HID, HPC * DQ))
        swq = np.float32(max(np.abs(wqT_f).max() / 127.0, 1e-30))
        wqT = np.clip(np.round(wqT_f / swq), -127, 127).astype(np.int8)
        wbnT = np.ascontiguousarray(
            Wkvbh[h0:h0 + HPC, :DN].transpose(2, 0, 1).reshape(KV, HPC * DN)).astype(bf)
        wbvT = np.ascontiguousarray(
            Wkvbh[h0:h0 + HPC, DN:].transpose(2, 0, 1).reshape(KV, HPC * DV)).astype(bf)
        wg3, wu3, wd3, sg2, su2, sd2 = _mlp_slices(core)
        wo_f = np.ascontiguousarray(WoT_f[h0 * DV:(h0 + HPC) * DV])
        swo = np.float32(max(np.abs(wo_f).max() / 127.0, 1e-30))
        woq8 = np.clip(np.round(wo_f / swo), -127, 127).astype(np.int8)
        sl = slice(core * TL, (core + 1) * TL)
        in_maps.append({
            "hid": hid_flat[sl],
            "wqT": wqT,
            "wkvaT": wkvaT,
            "wbnT": wbnT,
            "wbvT": wbvT,
            "woT": woq8,
            "satt": np.broadcast_to(
                np.array([swq, skv, swo], np.float32), (128, 3)).copy(),
            "wg3": wg3,
            "wu3": wu3,
            "wd3": wd3,
            "sg": sg2,
            "su": su2,
            "sd": sd2,
            "cosL": np.ascontiguousarray(cos_f[sl]),
            "sinL": np.ascontiguousarray(sin_f[sl]),
        })
    return in_maps


def run_cfg(cfg, nc, inputs_dict):
    from concourse.bass_utils import run_bass_kernel_spmd
    c = _derived(cfg)
    in_maps = prep_inputs(cfg, **inputs_dict)
    res = run_bass_kernel_spmd(nc, in_maps, list(range(cfg["N_CORES"])))
    out = np.concatenate(
        [res.results[i]["out"] for i in range(cfg["N_CORES"])], axis=0)
    return out.reshape(cfg["B"], cfg["S"], cfg["HID"]).astype(np.float32), res


_NC_CACHE = {}


def kernel(hidden_states, position_ids, Wq, Wkva, w_kvln, Wkvb, Wo, Wg, Wu, Wd,
           w_ln1, w_ln2):
    cfg = FULL_CFG
    if "full" not in _NC_CACHE:
        _NC_CACHE["full"] = build_kernel(cfg)
    out, _ = run_cfg(cfg, _NC_CACHE["full"], dict(
        hidden_states=np.asarray(hidden_states, np.float32),
        position_ids=np.asarray(position_ids),
        Wq=np.asarray(Wq, np.float32), Wkva=np.asarray(Wkva, np.float32),
        w_kvln=np.asarray(w_kvln, np.float32), Wkvb=np.asarray(Wkvb, np.float32),
        Wo=np.asarray(Wo, np.float32), Wg=np.asarray(Wg, np.float32),
        Wu=np.asarray(Wu, np.float32), Wd=np.asarray(Wd, np.float32),
        w_ln1=np.asarray(w_ln1, np.float32), w_ln2=np.asarray(w_ln2, np.float32)))
    return out



# revision 50
# speedup vs baseline: 1.1758x; 1.0903x over previous
"""DeepseekV2-Lite decoder layer on 8 Trainium2 NeuronCores.

Sharding: attention is tensor-parallel over heads (2 heads/core, all tokens);
o_proj is row-parallel; the MLP is tensor-parallel over the intermediate dim
(1368 cols/core, int8 weights + runtime scales) so gate/up/down weights are
sharded 8x instead of replicated. Three collectives total: AllGather of
(x_norm^T, c_norm^T, k_pe^T, cos^T, sin^T, token-major hid), AllReduce of
o_proj partials (giving every core all-token x2 inputs for the MLP), and a
ReduceScatter of down_proj partials with x2/8 folded in so its output IS the
finished layer output. Matmuls run in bf16 with fp32 PSUM accumulation.
"""
import math
import sys

sys.path.insert(0, "/opt/trn_rl_repo")

import numpy as np
import ml_dtypes

import concourse.bass as bass
import concourse.mybir as mybir
import concourse.tile as tile
from concourse.masks import make_identity

# ---------------------------------------------------------------------------
# Patch: the hardware CTRL instruction supports only one sync-wait slot, but
# kernels with collectives need several on the final Tile drain. Split the
# excess onto SP nops emitted right after the drain, before the sem-clear.
# ---------------------------------------------------------------------------
from concourse.vector_clock import ScopedClock


def _drain_and_barrier_split(self, tick_clock, wait_clock):
    drain_inst = self.nc.sync.drain()
    wait_clock.add_sem_waits(
        drain_inst.ins, ScopedClock({None: tick_clock.global_clock})
    )
    si = drain_inst.ins.sync_info
    if si is not None and len(si.on_wait) > 1:
        waits = list(si.on_wait)
        drain_inst.ins.sync_info = mybir.SyncInfo(
            on_wait=waits[:1], on_update=list(si.on_update)
        )
        for w in waits[1:]:
            nop = self.nc.sync.nop(nofuse=True, hint="drain_wait_overflow")
            nop.ins.sync_info = mybir.SyncInfo(on_wait=[w], on_update=[])
    self.nc.all_engine_barrier()
    assert self.sems is not None
    popped = self.nc._tile_sem_poison_stack.pop()
    assert popped is self._sem_poison
    self.nc.clear_and_free_semaphores(list(self.sems.allocated().values()))
    self.nc.all_engine_barrier()


tile.TileContext._drain_and_barrier = _drain_and_barrier_split

# ---------------------------------------------------------------------------
# Several instruction encodings (DMA, CTRL) accept only one sync-wait slot.
# Split every multi-wait instruction at BIR-serialization time: excess waits
# move onto same-engine NoOps inserted immediately before the instruction.
# ---------------------------------------------------------------------------
import orjson as _orjson

if not getattr(bass.Bass, "_wait_split_patched", False):
    bass.Bass._orig_to_json_bytes = bass.Bass.to_json_bytes
    bass.Bass._wait_split_patched = True
_orig_to_json_bytes = bass.Bass._orig_to_json_bytes


def _to_json_bytes_split(self):
    data = _orjson.loads(_orig_to_json_bytes(self))
    ctr = 0
    for f in data.get("functions", []):
        for bb in f.get("basic_blocks", f.get("blocks", [])):
            insts = bb.get("instructions", [])
            out = []
            for inst in insts:
                si = inst.get("sync_info")
                if si and len(si.get("on_wait") or []) > 1:
                    waits = si["on_wait"]
                    for w in waits[:-1]:
                        ctr += 1
                        out.append({
                            "debug": inst.get("debug", 0),
                            "engine": inst["engine"],
                            "ins": [], "name": f"I-ws{ctr}",
                            "opcode": "NoOp", "outs": [],
                            "sync_info": {"on_update": [], "on_wait": [w]},
                            "text_hint": "wait_split",
                        })
                    si["on_wait"] = [waits[-1]]
                out.append(inst)
            bb["instructions"] = out
    return _orjson.dumps(data)


bass.Bass.to_json_bytes = _to_json_bytes_split

# ---------------------------------------------------------------------------
FULL_CFG = dict(
    B=2, S=2048, HID=2048, H=16, D_NOPE=128, D_ROPE=64, D_V=128, KV=512,
    INTER=10944, N_CORES=8,
)
EPS = 1e-6
MAX_POS, BASE, FACTOR, ORIG_MAX = 8192, 10000.0, 40.0, 4096
BETA_FAST, BETA_SLOW, MSCALE, MSCALE_ALL = 32, 1, 0.707, 0.707

BF = mybir.dt.bfloat16
F32 = mybir.dt.float32
I8 = mybir.dt.int8
F16 = mybir.dt.float16
AX = mybir.AxisListType
AF = mybir.ActivationFunctionType


def _derived(cfg):
    d = dict(cfg)
    d["T_TOT"] = cfg["B"] * cfg["S"]
    d["T_LOC"] = d["T_TOT"] // cfg["N_CORES"]
    d["HPC"] = cfg["H"] // cfg["N_CORES"]
    d["KH"] = cfg["HID"] // 128
    d["KC"] = cfg["KV"] // 128
    d["TSUB"] = d["T_LOC"] // 128
    d["NCH"] = d["T_TOT"] // d["T_LOC"]
    d["ILOC"] = cfg["INTER"] // cfg["N_CORES"]   # 1368 intermediate cols/core
    d["ICL"] = (d["ILOC"] + 127) // 128          # 11 padded k-tiles/core
    d["IPAD"] = d["ICL"] * 128                   # 1408
    d["QTILES_B"] = cfg["S"] // 512
    d["KB_B"] = cfg["S"] // 128
    d["DQ"] = cfg["D_NOPE"] + cfg["D_ROPE"]
    # xnT + cnT + kpeT + cosT + sinT + flat token-major hid
    d["AGROWS"] = 2 * cfg["HID"] + cfg["KV"] + 2 * cfg["D_ROPE"]
    return d


# ---------------------------------------------------------------------------
def build_kernel(cfg):
    c = _derived(cfg)
    N = c["N_CORES"]
    HID, KV, DR, DN, DV = c["HID"], c["KV"], c["D_ROPE"], c["D_NOPE"], c["D_V"]
    TL, TT = c["T_LOC"], c["T_TOT"]
    KH, KC, TSUB, NCH, ICL = c["KH"], c["KC"], c["TSUB"], c["NCH"], c["ICL"]
    HPC, DQ = c["HPC"], c["DQ"]
    QT_B, KB_B = c["QTILES_B"], c["KB_B"]
    B = c["B"]
    HR = DR // 2
    AGR = c["AGROWS"]

    nc = bass.Bass()
    hid_e = nc.dram_tensor("hid", [TL, HID], BF, kind="ExternalInput")
    wqT_e = nc.dram_tensor("wqT", [HID, HPC * DQ], I8, kind="ExternalInput")
    wkvaT_e = nc.dram_tensor("wkvaT", [HID, KV + DR], I8, kind="ExternalInput")
    wbnT_e = nc.dram_tensor("wbnT", [KV, HPC * DN], I8, kind="ExternalInput")
    wbvT_e = nc.dram_tensor("wbvT", [KV, HPC * DV], I8, kind="ExternalInput")
    woT_e = nc.dram_tensor("woT", [HPC * DV, HID], I8, kind="ExternalInput")
    wg_e = nc.dram_tensor("wg3", [ICL, 128, KH, 128], I8, kind="ExternalInput")
    wu_e = nc.dram_tensor("wu3", [ICL, 128, KH, 128], I8, kind="ExternalInput")
    wd_e = nc.dram_tensor("wd3", [ICL, 128, HID], I8, kind="ExternalInput")
    satt_e = nc.dram_tensor("satt", [128, 5], F32, kind="ExternalInput")
    sg_e = nc.dram_tensor("sg", [128, ICL], F32, kind="ExternalInput")
    su_e = nc.dram_tensor("su", [128, ICL], F32, kind="ExternalInput")
    sd_e = nc.dram_tensor("sd", [128, ICL], F32, kind="ExternalInput")
    cosL_e = nc.dram_tensor("cosL", [TL, HR], F32, kind="ExternalInput")
    sinL_e = nc.dram_tensor("sinL", [TL, HR], F32, kind="ExternalInput")
    out_e = nc.dram_tensor("out", [TL, HID], F16, kind="ExternalOutput")
    probe = cfg.get("probe", False)
    if probe:
        p_agin_e = nc.dram_tensor("p_agin", [AGR, TL], BF, kind="ExternalOutput")

    with tile.TileContext(nc) as tc:
        with (
            tc.tile_pool(name="dram", bufs=1, space="DRAM") as dram,
            tc.tile_pool(name="const", bufs=1) as const,
        ):
            agin = dram.tile([AGR, TL], BF, tag="agin", name="agin")
            agout = dram.tile([N * AGR, TL], BF, addr_space="Shared", tag="agout", name="agout")
            rs_in = dram.tile([TT, HID], F16, tag="rsin", name="rsin")
            x2a = dram.tile([TT, HID], F16, addr_space="Shared", tag="x2a", name="x2a")
            rs2_in = dram.tile([TT, HID], F16, tag="rs2in", name="rs2in")
            rs2_out = dram.tile([TL, HID], F16, tag="rs2out", name="rs2out")
            OFF_KPE = HID + KV
            OFF_COS = OFF_KPE + DR
            OFF_SIN = OFF_COS + HR
            OFF_HID = OFF_SIN + HR

            ident = const.tile([128, 128], BF, tag="ident", name="ident")
            make_identity(nc, ident)
            eps_sb = const.tile([128, 1], F32, tag="eps", name="eps")
            nc.vector.memset(eps_sb[:], EPS)
            # mask[p, x] = 1.0 where x >= p + 384, else 0 — generated on device
            mask_sb = const.tile([128, 896], BF, tag="mask", name="mask")
            nc.gpsimd.memset(mask_sb[:], 1.0)
            nc.gpsimd.affine_select(
                out=mask_sb[:], in_=mask_sb[:],
                compare_op=mybir.AluOpType.is_ge, fill=0.0,
                base=-384, pattern=[[1, 896]], channel_multiplier=-1)
            cosL_sb = const.tile([128, TSUB, HR], F32, tag="cosL", name="cosL")
            nc.sync.dma_start(cosL_sb[:], cosL_e.rearrange("(a p) r -> p a r", p=128))
            sinL_sb = const.tile([128, TSUB, HR], F32, tag="sinL", name="sinL")
            nc.sync.dma_start(sinL_sb[:], sinL_e.rearrange("(a p) r -> p a r", p=128))
            satt_sb = const.tile([128, 5], F32, tag="satt", name="satt")
            nc.sync.dma_start(satt_sb[:], satt_e[:])
            sg_sb = const.tile([128, ICL], F32, tag="sg", name="sg")
            nc.sync.dma_start(sg_sb[:], sg_e[:])
            su_sb = const.tile([128, ICL], F32, tag="su", name="su")
            nc.sync.dma_start(su_sb[:], su_e[:])
            sd_sb = const.tile([128, ICL], F32, tag="sd", name="sd")
            nc.sync.dma_start(sd_sb[:], sd_e[:])

            # ============ phases 0-1: rms1, x^T, ckv, rms(c), rope(k_pe) =====
            with (
                tc.tile_pool(name="xnTp", bufs=1) as xnTp,
                tc.tile_pool(name="p0", bufs=2) as p0,
                tc.tile_pool(name="p01ps", bufs=2, space="PSUM") as p01ps,
            ):
                xnT = [xnTp.tile([128, TL], BF, tag=f"xnT{k}", name=f"xnT{k}") for k in range(KH)]
                # token-major hid rides the AllGather as a flat [TL*HID/TL, TL] region
                nc.sync.dma_start(
                    agin[OFF_HID:OFF_HID + HID, :],
                    hid_e.rearrange("t (a c) -> (t a) c", c=TL))
                xn_sb = []
                for t in range(TSUB):
                    ht = p0.tile([128, HID], BF, tag="hid0", name="hid0")
                    nc.sync.dma_start(ht[:], hid_e[t * 128:(t + 1) * 128, :])
                    sq = p0.tile([128, HID], F32, tag="sq", name="sq")
                    nc.vector.tensor_mul(sq[:], ht[:], ht[:])
                    ssum = p0.tile([128, 1], F32, tag="ssum", name="ssum")
                    nc.vector.reduce_sum(out=ssum[:], in_=sq[:], axis=AX.X)
                    rs = p0.tile([128, 1], F32, tag="rs", name="rs")
                    nc.scalar.activation(rs[:], ssum[:], AF.Sqrt, scale=1.0 / HID, bias=eps_sb[:])
                    nc.vector.reciprocal(rs[:], rs[:])
                    xt = p0.tile([128, HID], BF, tag="xn", name="xn", bufs=TSUB)
                    nc.vector.tensor_scalar_mul(xt[:], ht[:], rs[:])
                    xn_sb.append(xt)
                for t in range(TSUB):
                    for k in range(KH):
                        ps = p01ps.tile([128, 128], BF, tag="tr", name="tr")
                        nc.tensor.transpose(ps[:], xn_sb[t][:, k * 128:(k + 1) * 128], ident[:])
                        nc.scalar.copy(xnT[k][:, t * 128:(t + 1) * 128], ps[:])
                for k in range(KH):
                    nc.sync.dma_start(agin[k * 128:(k + 1) * 128, :], xnT[k][:])

                # phase 1
                wkva_sb = [p0.tile([128, KV + DR], BF, tag=f"wkva{k}", name=f"wkva{k}") for k in range(KH)]
                for k in range(KH):
                    wkq = p0.tile([128, KV + DR], I8, tag="wkq", name="wkq")
                    nc.sync.dma_start(wkq[:], wkvaT_e[k * 128:(k + 1) * 128, :])
                    nc.scalar.copy(wkva_sb[k][:], wkq[:])
                # cos/sin scaled by s_kv for the k_pe rope (k_pe psum is raw int units)
                clk = p0.tile([128, TSUB, HR], F32, tag="clk", name="clk", bufs=1)
                nc.vector.tensor_scalar_mul(clk[:], cosL_sb[:], satt_sb[:, 1:2])
                slk = p0.tile([128, TSUB, HR], F32, tag="slk", name="slk", bufs=1)
                nc.vector.tensor_scalar_mul(slk[:], sinL_sb[:], satt_sb[:, 1:2])
                cnT_sb = [p0.tile([128, TL], BF, tag=f"cnT{j}", name=f"cnT{j}") for j in range(KC)]
                kpeT_loc = p0.tile([DR, TL], BF, tag="kpeT_loc", name="kpeT_loc")
                for t in range(TSUB):
                    ps_c = p01ps.tile([128, KV], F32, tag="psc", name="psc")
                    ps_p = p01ps.tile([128, DR], F32, tag="psp", name="psp")
                    for k in range(KH):
                        lq = xnT[k][:, t * 128:(t + 1) * 128]
                        nc.tensor.matmul(ps_c[:], lq, wkva_sb[k][:, :KV],
                                         start=(k == 0), stop=(k == KH - 1))
                        nc.tensor.matmul(ps_p[:], lq, wkva_sb[k][:, KV:],
                                         start=(k == 0), stop=(k == KH - 1))
                    sq = p0.tile([128, KV], F32, tag="sqc", name="sqc")
                    nc.scalar.activation(sq[:], ps_c[:], AF.Square)
                    ssum = p0.tile([128, 1], F32, tag="ssumc", name="ssumc")
                    nc.vector.reduce_sum(out=ssum[:], in_=sq[:], axis=AX.X)
                    rs = p0.tile([128, 1], F32, tag="rsc", name="rsc")
                    nc.scalar.activation(rs[:], ssum[:], AF.Sqrt, scale=1.0 / KV, bias=eps_sb[:])
                    nc.vector.reciprocal(rs[:], rs[:])
                    cn = p0.tile([128, KV], BF, tag="cn", name="cn")
                    nc.vector.tensor_scalar_mul(cn[:], ps_c[:], rs[:])
                    kp = p0.tile([128, DR], BF, tag="kp", name="kp")
                    a = p0.tile([128, HR], F32, tag="ra", name="ra")
                    b = p0.tile([128, HR], F32, tag="rb", name="rb")
                    cosl = clk[:, t, :]
                    sinl = slk[:, t, :]
                    nc.vector.tensor_mul(a[:], ps_p[:, :HR], cosl)
                    nc.vector.tensor_mul(b[:], ps_p[:, HR:], sinl)
                    nc.vector.tensor_sub(kp[:, :HR], a[:], b[:])
                    nc.vector.tensor_mul(a[:], ps_p[:, HR:], cosl)
                    nc.vector.tensor_mul(b[:], ps_p[:, :HR], sinl)
                    nc.vector.tensor_add(kp[:, HR:], a[:], b[:])
                    for j in range(KC):
                        ps = p01ps.tile([128, 128], BF, tag="tr", name="tr")
                        nc.tensor.transpose(ps[:], cn[:, j * 128:(j + 1) * 128], ident[:])
                        nc.scalar.copy(cnT_sb[j][:, t * 128:(t + 1) * 128], ps[:])
                    ps = p01ps.tile([128, 128], BF, tag="tr", name="tr")
                    nc.tensor.transpose(ps[:DR, :], kp[:], ident[:])
                    nc.scalar.copy(kpeT_loc[:, t * 128:(t + 1) * 128], ps[:DR, :])
                for j in range(KC):
                    nc.sync.dma_start(agin[HID + j * 128:HID + (j + 1) * 128, :], cnT_sb[j][:])
                nc.sync.dma_start(agin[OFF_KPE:OFF_KPE + DR, :], kpeT_loc[:])
                # ride local cos/sin (transposed, bf16) for the q-rope phase
                cl_bf = p0.tile([128, TSUB, HR], BF, tag="clbf", name="clbf", bufs=1)
                nc.vector.tensor_scalar_mul(cl_bf[:], cosL_sb[:], satt_sb[:, 0:1])
                sl_bf = p0.tile([128, TSUB, HR], BF, tag="slbf", name="slbf", bufs=1)
                nc.vector.tensor_scalar_mul(sl_bf[:], sinL_sb[:], satt_sb[:, 0:1])
                cosT_loc = p0.tile([HR, TL], BF, tag="cosTl", name="cosTl", bufs=1)
                sinT_loc = p0.tile([HR, TL], BF, tag="sinTl", name="sinTl", bufs=1)
                for t in range(TSUB):
                    ps = p01ps.tile([128, 128], BF, tag="tr", name="tr")
                    nc.tensor.transpose(ps[:HR, :], cl_bf[:, t, :], ident[:])
                    nc.scalar.copy(cosT_loc[:, t * 128:(t + 1) * 128], ps[:HR, :])
                    ps = p01ps.tile([128, 128], BF, tag="tr", name="tr")
                    nc.tensor.transpose(ps[:HR, :], sl_bf[:, t, :], ident[:])
                    nc.scalar.copy(sinT_loc[:, t * 128:(t + 1) * 128], ps[:HR, :])
                nc.sync.dma_start(agin[OFF_COS:OFF_COS + HR, :], cosT_loc[:])
                nc.sync.dma_start(agin[OFF_SIN:OFF_SIN + HR, :], sinT_loc[:])

            # ============ phase 2: AllGather ================================
            nc.gpsimd.collective_compute(
                "AllGather", mybir.AluOpType.bypass,
                replica_groups=[list(range(N))],
                ins=[agin.opt()], outs=[agout.opt()],
            )

            if probe:
                with tc.tile_pool(name="prb0", bufs=2) as prb0:
                    for r in range(0, AGR, 128):
                        w = min(128, AGR - r)
                        pt_ = prb0.tile([128, TL], BF, tag="pgt", name="pgt")
                        nc.sync.dma_start(pt_[:w, :], agin[r:r + w, :])
                        nc.sync.dma_start(p_agin_e[r:r + w, :], pt_[:w, :])

            # ============ phases 3-5: attention ==============================
            with tc.tile_pool(name="asb", bufs=1) as asb:
                qnT = [asb.tile([128, TT], BF, tag=f"qnT{h}", name=f"qnT{h}") for h in range(HPC)]
                qpT = [asb.tile([DR, TT], BF, tag=f"qpT{h}", name=f"qpT{h}") for h in range(HPC)]
                knT = [asb.tile([128, TT], BF, tag=f"knT{h}", name=f"knT{h}") for h in range(HPC)]
                kpeT = asb.tile([DR, TT], BF, tag="kpeT", name="kpeT")
                v_sb = [asb.tile([128, TT // 128, DV + 4], BF, tag=f"v{h}", name=f"v{h}")
                        for h in range(HPC)]
                atT = [asb.tile([128, TT], BF, tag=f"atT{h}", name=f"atT{h}") for h in range(HPC)]
                cosT_sb = asb.tile([HR, TT], BF, tag="cosT", name="cosT")
                sinT_sb = asb.tile([HR, TT], BF, tag="sinT", name="sinT")

                with (
                    tc.tile_pool(name="p4w", bufs=1) as p4w,
                    tc.tile_pool(name="p4x", bufs=1) as p4x,
                    tc.tile_pool(name="p4", bufs=2) as p4,
                    tc.tile_pool(name="p4ps", bufs=2, space="PSUM") as p4ps,
                ):
                    wq_sb = [p4w.tile([128, HPC * DQ], BF, tag=f"wq{k}", name=f"wq{k}") for k in range(KH)]
                    for k in range(KH):
                        wqq = p4.tile([128, HPC * DQ], I8, tag="wqq", name="wqq")
                        nc.sync.dma_start(wqq[:], wqT_e[k * 128:(k + 1) * 128, :])
                        nc.scalar.copy(wq_sb[k][:], wqq[:])
                    wbn_sb = [p4w.tile([128, HPC * DN], BF, tag=f"wbn{j}", name=f"wbn{j}") for j in range(KC)]
                    wbv_sb = [p4w.tile([128, HPC * DV], BF, tag=f"wbv{j}", name=f"wbv{j}") for j in range(KC)]
                    for j in range(KC):
                        wbnq = p4.tile([128, HPC * DN], I8, tag="wbnq", name="wbnq")
                        nc.sync.dma_start(wbnq[:], wbnT_e[j * 128:(j + 1) * 128, :])
                        nc.scalar.copy(wbn_sb[j][:], wbnq[:])
                        wbvq = p4.tile([128, HPC * DV], I8, tag="wbvq", name="wbvq")
                        nc.sync.dma_start(wbvq[:], wbvT_e[j * 128:(j + 1) * 128, :])
                        nc.scalar.copy(wbv_sb[j][:], wbvq[:])

                    for ch in range(NCH):
                        nc.sync.dma_start(
                            kpeT[:, ch * TL:(ch + 1) * TL],
                            agout[ch * AGR + OFF_KPE: ch * AGR + OFF_KPE + DR, :])
                        nc.sync.dma_start(
                            cosT_sb[:, ch * TL:(ch + 1) * TL],
                            agout[ch * AGR + OFF_COS: ch * AGR + OFF_COS + HR, :])
                        nc.sync.dma_start(
                            sinT_sb[:, ch * TL:(ch + 1) * TL],
                            agout[ch * AGR + OFF_SIN: ch * AGR + OFF_SIN + HR, :])

                    for ch in range(NCH):
                        xch = []
                        for k in range(KH):
                            xt = p4x.tile([128, TL], BF, tag="xch", name="xch", bufs=KH + 4)
                            nc.sync.dma_start(
                                xt[:], agout[ch * AGR + k * 128: ch * AGR + (k + 1) * 128, :])
                            xch.append(xt)
                        cs = slice(ch * TL, (ch + 1) * TL)
                        for h in range(HPC):
                            ps_n = p4ps.tile([128, TL], F32, tag="qn", name="qn")
                            ps_p = p4ps.tile([DR, TL], F32, tag="qp", name="qp")
                            off = h * DQ
                            for k in range(KH):
                                nc.tensor.matmul(ps_n[:], wq_sb[k][:, off:off + DN], xch[k][:],
                                                 start=(k == 0), stop=(k == KH - 1))
                            for k in range(KH):
                                nc.tensor.matmul(ps_p[:], wq_sb[k][:, off + DN:off + DQ], xch[k][:],
                                                 start=(k == 0), stop=(k == KH - 1))
                            nc.vector.tensor_scalar_mul(qnT[h][:, cs], ps_n[:], satt_sb[:, 0:1])
                            a = p4.tile([HR, TL], F32, tag="qa", name="qa")
                            b = p4.tile([HR, TL], F32, tag="qb", name="qb")
                            cosc = cosT_sb[:, cs]
                            sinc = sinT_sb[:, cs]
                            nc.vector.tensor_mul(a[:], ps_p[:HR, :], cosc)
                            nc.vector.tensor_mul(b[:], ps_p[HR:, :], sinc)
                            nc.vector.tensor_sub(qpT[h][:HR, cs], a[:], b[:])
                            nc.vector.tensor_mul(a[:], ps_p[HR:, :], cosc)
                            nc.vector.tensor_mul(b[:], ps_p[:HR, :], sinc)
                            nc.vector.tensor_add(qpT[h][HR:, cs], a[:], b[:])

                    for ch in range(NCH):
                        cch = []
                        for j in range(KC):
                            ct = p4x.tile([128, TL], BF, tag="cch", name="cch", bufs=KC + 2)
                            nc.sync.dma_start(
                                ct[:], agout[ch * AGR + HID + j * 128: ch * AGR + HID + (j + 1) * 128, :])
                            cch.append(ct)
                        cs = slice(ch * TL, (ch + 1) * TL)
                        for h in range(HPC):
                            ps_k = p4ps.tile([128, TL], F32, tag="kn", name="kn")
                            for j in range(KC):
                                nc.tensor.matmul(ps_k[:], wbn_sb[j][:, h * DN:(h + 1) * DN], cch[j][:],
                                                 start=(j == 0), stop=(j == KC - 1))
                            nc.vector.tensor_scalar_mul(knT[h][:, cs], ps_k[:], satt_sb[:, 3:4])
                            for j4 in range(TL // 128):
                                ps_v = p4ps.tile([128, DV], F32, tag="pv", name="pv")
                                for j in range(KC):
                                    nc.tensor.matmul(ps_v[:], cch[j][:, j4 * 128:(j4 + 1) * 128],
                                                     wbv_sb[j][:, h * DV:(h + 1) * DV],
                                                     start=(j == 0), stop=(j == KC - 1))
                                kbt = ch * (TL // 128) + j4
                                nc.vector.tensor_scalar_mul(v_sb[h][:, kbt, :DV], ps_v[:], satt_sb[:, 4:5])
                                nc.vector.memset(v_sb[h][:, kbt, DV:DV + 1], 1.0)

                # ---------------- phase 5: attention -------------------------
                with (
                    tc.tile_pool(name="p5ps", bufs=2, space="PSUM") as p5ps,
                    tc.tile_pool(name="p5pv", bufs=2, space="PSUM") as p5pv,
                    tc.tile_pool(name="p5", bufs=2) as p5,
                    tc.tile_pool(name="prb", bufs=1) as prb,
                ):
                    for b in range(B):
                        for h in range(HPC):
                            for qt in range(QT_B):
                                qs = slice(b * cfg["S"] + qt * 512, b * cfg["S"] + qt * 512 + 512)
                                nkb = 4 * qt + 4
                                pt = []
                                for kb in range(nkb):
                                    kbg = b * KB_B + kb
                                    ks = slice(kbg * 128, kbg * 128 + 128)
                                    ps_s = p5ps.tile([128, 512], F32, tag="ps_s", name="ps_s")
                                    nc.tensor.matmul(ps_s[:], knT[h][:, ks], qnT[h][:, qs],
                                                     start=True, stop=False)
                                    nc.tensor.matmul(ps_s[:], kpeT[:, ks], qpT[h][:, qs],
                                                     start=False, stop=True)
                                    pb = prb.tile([128, 512], BF, tag="pb", name="pb", bufs=KB_B + 4)
                                    nc.scalar.activation(pb[:], ps_s[:], AF.Exp)
                                    delta = kb * 128 - qt * 512
                                    if delta >= 0:
                                        nc.vector.tensor_mul(
                                            pb[:], pb[:], mask_sb[:, 384 - delta:896 - delta])
                                    pt.append(pb)
                                for q4 in range(4):
                                    ps_av = p5pv.tile([128, DV + 4], F32, tag="ps_av", name="ps_av")
                                    for kb in range(nkb):
                                        kbt = b * KB_B + kb
                                        nc.tensor.matmul(
                                            ps_av[:, :DV + 1],
                                            pt[kb][:, q4 * 128:(q4 + 1) * 128],
                                            v_sb[h][:, kbt, :DV + 1],
                                            start=(kb == 0), stop=(kb == nkb - 1))
                                    recip = p5.tile([128, 1], F32, tag="recip", name="recip")
                                    nc.vector.reciprocal(recip[:], ps_av[:, DV:DV + 1])
                                    at = p5.tile([128, DV], BF, tag="at", name="at")
                                    nc.vector.tensor_scalar_mul(at[:], ps_av[:, :DV], recip[:])
                                    ps_t = p5ps.tile([128, 128], BF, tag="ps_t", name="ps_t")
                                    nc.tensor.transpose(ps_t[:DV, :], at[:], ident[:])
                                    qg = (b * cfg["S"] + qt * 512) // 128 + q4
                                    nc.scalar.copy(atT[h][:DV, qg * 128:(qg + 1) * 128], ps_t[:DV, :])

                # ============ phase 5b: row-parallel o_proj partials =============
                with (
                    tc.tile_pool(name="p6w", bufs=1) as p6w,
                    tc.tile_pool(name="p6", bufs=4) as p6,
                    tc.tile_pool(name="p6ps", bufs=4, space="PSUM") as p6ps,
                ):
                    wo_sb = [p6w.tile([128, HID], BF, tag=f"wo{j}", name=f"wo{j}") for j in range(HPC)]
                    for j in range(HPC):
                        woq = p6.tile([128, HID], I8, tag="woq", name="woq")
                        nc.sync.dma_start(woq[:DV, :], woT_e[j * DV:(j + 1) * DV, :])
                        nc.scalar.copy(wo_sb[j][:DV, :], woq[:DV, :])
                    for tq in range(TT // 128):
                        for nsl in range(HID // 512):
                            ps_o = p6ps.tile([128, 512], F32, tag="ps_o", name="ps_o")
                            for j in range(HPC):
                                nc.tensor.matmul(ps_o[:], atT[j][:DV, tq * 128:(tq + 1) * 128],
                                                 wo_sb[j][:, nsl * 512:(nsl + 1) * 512],
                                                 start=(j == 0), stop=(j == HPC - 1))
                            ob = p6.tile([128, 512], F16, tag="ob", name="ob")
                            nc.vector.tensor_scalar_mul(ob[:], ps_o[:], satt_sb[:, 2:3])
                            nc.sync.dma_start(
                                rs_in[tq * 128:(tq + 1) * 128, nsl * 512:(nsl + 1) * 512], ob[:])

            # ============ phase 6: AllReduce o_proj partials ================
            nc.gpsimd.collective_compute(
                "AllReduce", mybir.AluOpType.add,
                replica_groups=[list(range(N))],
                ins=[rs_in.opt()], outs=[x2a.opt()],
            )

            # ============ phases 7-8: x2, rms2, TP MLP over INTER ============
            # Every core: for each 512-token chunk, assemble x2 = o_attn + hid
            # (both all-token), rms2 + transpose to y^T, gate/up/down on its
            # 1368-col INTER slice, fold x2/8 into the down partials so the
            # final ReduceScatter(add) emits the finished layer output.
            with (
                tc.tile_pool(name="p8wd", bufs=1) as p8wd,
                tc.tile_pool(name="p8w", bufs=2) as p8w,
                tc.tile_pool(name="p8x", bufs=2) as p8x,
                tc.tile_pool(name="p8sq", bufs=2) as p8sq,
                tc.tile_pool(name="p8y", bufs=1) as p8y,
                tc.tile_pool(name="p8h", bufs=1) as p8h,
                tc.tile_pool(name="p8", bufs=4) as p8,
                tc.tile_pool(name="p8ps", bufs=2, space="PSUM") as p8ps,
                tc.tile_pool(name="p8psd", bufs=2, space="PSUM") as p8psd,
                tc.tile_pool(name="p8pst", bufs=2, space="PSUM") as p8pst,
            ):
                wd_sb = [p8wd.tile([128, HID], BF, tag=f"wd{i}", name=f"wd{i}")
                         for i in range(ICL)]
                for i in range(ICL):
                    wdq = p8w.tile([128, HID], I8, tag="wdq", name="wdq")
                    nc.sync.dma_start(wdq[:], wd_e[i])
                    nc.vector.tensor_scalar_mul(wd_sb[i][:], wdq[:], sd_sb[:, i:i + 1])
                for ch in range(NCH):
                    x2c, x2s = [], []
                    for t in range(TSUB):
                        oc = p8x.tile([128, HID], F16, tag="oc", name="oc")
                        nc.sync.dma_start(
                            oc[:], x2a[ch * TL + t * 128: ch * TL + (t + 1) * 128, :])
                        hc = p8x.tile([128, HID], BF, tag="hc", name="hc")
                        nc.sync.dma_start(
                            hc[:],
                            agout[ch * AGR + OFF_HID + t * 512:
                                  ch * AGR + OFF_HID + (t + 1) * 512, :]
                            .rearrange("(p a) c -> p (a c)", p=128))
                        xb = p8x.tile([128, HID], F16, tag="xc", name="xc", bufs=TSUB)
                        nc.vector.tensor_add(xb[:], oc[:], hc[:])
                        x2c.append(xb)
                        xs = p8x.tile([128, HID], F16, tag="xs", name="xs", bufs=TSUB)
                        nc.scalar.activation(xs[:], xb[:], AF.Copy, scale=0.125)
                        x2s.append(xs)
                    ynT = [p8y.tile([128, TL], BF, tag=f"ynT{k}", name=f"ynT{k}", bufs=1)
                           for k in range(KH)]
                    for t in range(TSUB):
                        sq = p8sq.tile([128, HID], F32, tag="sq", name="sq")
                        nc.vector.tensor_mul(sq[:], x2c[t][:], x2c[t][:])
                        ssum = p8.tile([128, 1], F32, tag="ssum", name="ssum")
                        nc.vector.reduce_sum(out=ssum[:], in_=sq[:], axis=AX.X)
                        rsc = p8.tile([128, 1], F32, tag="rsc", name="rsc")
                        nc.scalar.activation(rsc[:], ssum[:], AF.Sqrt, scale=1.0 / HID, bias=eps_sb[:])
                        nc.vector.reciprocal(rsc[:], rsc[:])
                        yt = p8.tile([128, HID], BF, tag="yn", name="yn", bufs=2)
                        nc.vector.tensor_scalar_mul(yt[:], x2c[t][:], rsc[:])
                        for k in range(KH):
                            ps = p8pst.tile([128, 128], BF, tag="tr", name="tr")
                            nc.tensor.transpose(ps[:], yt[:, k * 128:(k + 1) * 128], ident[:])
                            nc.scalar.copy(ynT[k][:, t * 128:(t + 1) * 128], ps[:])
                    hT = []
                    for i in range(ICL):
                        wgq = p8w.tile([128, KH, 128], I8, tag="wgq", name="wgq")
                        nc.sync.dma_start(wgq[:], wg_e[i])
                        wg_sb = p8w.tile([128, KH, 128], BF, tag="wg", name="wg")
                        nc.scalar.copy(wg_sb[:], wgq[:])
                        wuq = p8w.tile([128, KH, 128], I8, tag="wuq", name="wuq")
                        nc.sync.dma_start(wuq[:], wu_e[i])
                        wu_sb = p8w.tile([128, KH, 128], BF, tag="wu", name="wu")
                        nc.scalar.copy(wu_sb[:], wuq[:])
                        ps_g = p8ps.tile([128, TL], F32, tag="psg", name="psg")
                        ps_u = p8ps.tile([128, TL], F32, tag="psu", name="psu")
                        for k in range(KH):
                            nc.tensor.matmul(ps_g[:], wg_sb[:, k, :], ynT[k][:],
                                             start=(k == 0), stop=(k == KH - 1))
                        for k in range(KH):
                            nc.tensor.matmul(ps_u[:], wu_sb[:, k, :], ynT[k][:],
                                             start=(k == 0), stop=(k == KH - 1))
                        sig = p8.tile([128, TL], BF, tag="sig", name="sig")
                        nc.scalar.activation(sig[:], ps_g[:], AF.Silu,
                                             scale=sg_sb[:, i:i + 1])
                        ub = p8.tile([128, TL], BF, tag="ub", name="ub")
                        nc.vector.tensor_scalar_mul(ub[:], ps_u[:], su_sb[:, i:i + 1])
                        ht = p8h.tile([128, TL], BF, tag="hT", name="hT", bufs=ICL + 2)
                        nc.vector.tensor_mul(ht[:], sig[:], ub[:])
                        hT.append(ht)
                    for tt in range(TSUB):
                        for ng in range(HID // 512):
                            ps_d = p8psd.tile([128, 512], F32, tag="psd", name="psd")
                            for i in range(ICL):
                                nc.tensor.matmul(ps_d[:], hT[i][:, tt * 128:(tt + 1) * 128],
                                                 wd_sb[i][:, ng * 512:(ng + 1) * 512],
                                                 start=(i == 0), stop=(i == ICL - 1))
                            ob = p8.tile([128, 512], F16, tag="ob", name="ob")
                            nc.vector.tensor_add(
                                ob[:], ps_d[:], x2s[tt][:, ng * 512:(ng + 1) * 512])
                            nc.sync.dma_start(
                                rs2_in[ch * TL + tt * 128: ch * TL + (tt + 1) * 128,
                                       ng * 512:(ng + 1) * 512], ob[:])

            # ============ phase 9: ReduceScatter -> finished output ==========
            nc.gpsimd.collective_compute(
                "ReduceScatter", mybir.AluOpType.add,
                replica_groups=[list(range(N))],
                ins=[rs2_in.opt()], outs=[rs2_out.opt()],
            )
            nc.sync.dma_start(out_e[:, :], rs2_out[:])
    return nc


# ---------------------------------------------------------------------------
# Host-side prep
# ---------------------------------------------------------------------------
def _yarn_tables(position_ids, d_rope):
    ar = np.arange(0, d_rope, 2, dtype=np.float32) / d_rope
    freq_extra = 1.0 / BASE ** ar
    freq_inter = 1.0 / (FACTOR * BASE ** ar)

    def corr_dim(num_rot):
        return d_rope * math.log(ORIG_MAX / (num_rot * 2 * math.pi)) / (2 * math.log(BASE))

    low = max(math.floor(corr_dim(BETA_FAST)), 0)
    high = min(math.ceil(corr_dim(BETA_SLOW)), d_rope - 1)
    hi = high + 0.001 if low == high else high
    ramp = np.clip((np.arange(d_rope // 2, dtype=np.float32) - low) / (hi - low), 0.0, 1.0)
    inv_freq_mask = 1.0 - ramp
    inv_freq = freq_inter * (1 - inv_freq_mask) + freq_extra * inv_freq_mask

    def get_mscale(s, m):
        return 1.0 if s <= 1 else 0.1 * m * math.log(s) + 1.0

    ms = get_mscale(FACTOR, MSCALE) / get_mscale(FACTOR, MSCALE_ALL)
    pos = np.asarray(position_ids).reshape(-1).astype(np.float32)
    fr = np.outer(pos, inv_freq)
    return (np.cos(fr) * ms).astype(np.float32), (np.sin(fr) * ms).astype(np.float32)


def _deint_perm(d):
    p = np.empty(d, np.int64)
    p[:d // 2] = 2 * np.arange(d // 2)
    p[d // 2:] = 2 * np.arange(d // 2) + 1
    return p


def prep_inputs(cfg, hidden_states, position_ids, Wq, Wkva, w_kvln, Wkvb, Wo,
                Wg, Wu, Wd, w_ln1, w_ln2):
    c = _derived(cfg)
    N, HPC = c["N_CORES"], c["HPC"]
    HID, KV, DR, DN, DV, DQ = c["HID"], c["KV"], c["D_ROPE"], c["D_NOPE"], c["D_V"], c["DQ"]
    TL, TT, KH = c["T_LOC"], c["T_TOT"], c["KH"]
    ILOC, ICL, IPAD = c["ILOC"], c["ICL"], c["IPAD"]
    bf = ml_dtypes.bfloat16

    hid_flat = np.ascontiguousarray(hidden_states.reshape(TT, HID)).astype(bf)
    perm = _deint_perm(DR)
    scale = np.float32(DQ ** -0.5)

    Wq = Wq * w_ln1[None, :] * scale
    Wqh = Wq.reshape(cfg["H"], DQ, HID)
    Wqh = np.concatenate([Wqh[:, :DN], Wqh[:, DN:][:, perm]], axis=1)
    Wkva = Wkva * w_ln1[None, :]
    Wkva = np.concatenate([Wkva[:KV], Wkva[KV:][perm]], axis=0)
    wkvaT_f = np.ascontiguousarray(Wkva.T)
    skv = np.float32(max(np.abs(wkvaT_f).max() / 127.0, 1e-30))
    wkvaT = np.clip(np.round(wkvaT_f / skv), -127, 127).astype(np.int8)
    Wkvb = Wkvb * w_kvln[None, :]
    Wkvbh = Wkvb.reshape(cfg["H"], DN + DV, KV)
    WoT_f = np.ascontiguousarray(Wo.T, dtype=np.float32)
    WgT_f = (Wg * w_ln2[None, :]).T          # [HID, INTER]
    WuT_f = (Wu * w_ln2[None, :]).T
    WdT_f = Wd.T                             # [INTER, HID]

    def _quant_cols(w):
        # per-column symmetric int8: w[:, i] = q[:, i] * s[i]
        s = np.abs(w).max(axis=0) / 127.0
        s[s == 0] = 1.0
        q = np.clip(np.round(w / s[None, :]), -127, 127).astype(np.int8)
        return q, s.astype(np.float32)

    def _mlp_slices(core):
        i0 = core * ILOC
        gc = np.zeros((HID, IPAD), np.float32)
        gc[:, :ILOC] = WgT_f[:, i0:i0 + ILOC]
        uc = np.zeros((HID, IPAD), np.float32)
        uc[:, :ILOC] = WuT_f[:, i0:i0 + ILOC]
        dc = np.zeros((IPAD, HID), np.float32)
        dc[:ILOC] = WdT_f[i0:i0 + ILOC]
        gq, sg = _quant_cols(gc)
        uq, su = _quant_cols(uc)
        dqT, sd = _quant_cols(dc.T)                          # per-row of dc
        dq = np.ascontiguousarray(dqT.T)
        wg3 = np.ascontiguousarray(gq.reshape(KH, 128, ICL, 128).transpose(2, 1, 0, 3))
        wu3 = np.ascontiguousarray(uq.reshape(KH, 128, ICL, 128).transpose(2, 1, 0, 3))
        wd3 = np.ascontiguousarray(dq.reshape(ICL, 128, HID))
        sg2 = np.ascontiguousarray(sg.reshape(ICL, 128).T)   # [i_inner, i_tile]
        su2 = np.ascontiguousarray(su.reshape(ICL, 128).T)
        sd2 = np.ascontiguousarray(sd.reshape(ICL, 128).T)
        return wg3, wu3, wd3, sg2, su2, sd2

    cos_f, sin_f = _yarn_tables(position_ids, DR)

    in_maps = []
    for core in range(N):
        h0 = core * HPC
        wqT_f = np.ascontiguousarray(
            Wqh[h0:h0 + HPC].transpose(2, 0, 1).reshape(HID, HPC * DQ))
        swq = np.float32(max(np.abs(wqT_f).max() / 127.0, 1e-30))
        wqT = np.clip(np.round(wqT_f / swq), -127, 127).astype(np.int8)
        wbnT_f = np.ascontiguousarray(
            Wkvbh[h0:h0 + HPC, :DN].transpose(2, 0, 1).reshape(KV, HPC * DN))
        sbn = np.float32(max(np.abs(wbnT_f).max() / 127.0, 1e-30))
        wbnT = np.clip(np.round(wbnT_f / sbn), -127, 127).astype(np.int8)
        wbvT_f = np.ascontiguousarray(
            Wkvbh[h0:h0 + HPC, DN:].transpose(2, 0, 1).reshape(KV, HPC * DV))
        sbv = np.float32(max(np.abs(wbvT_f).max() / 127.0, 1e-30))
        wbvT = np.clip(np.round(wbvT_f / sbv), -127, 127).astype(np.int8)
        wg3, wu3, wd3, sg2, su2, sd2 = _mlp_slices(core)
        wo_f = np.ascontiguousarray(WoT_f[h0 * DV:(h0 + HPC) * DV])
        swo = np.float32(max(np.abs(wo_f).max() / 127.0, 1e-30))
        woq8 = np.clip(np.round(wo_f / swo), -127, 127).astype(np.int8)
        sl = slice(core * TL, (core + 1) * TL)
        in_maps.append({
            "hid": hid_flat[sl],
            "wqT": wqT,
            "wkvaT": wkvaT,
            "wbnT": wbnT,
            "wbvT": wbvT,
            "woT": woq8,
            "satt": np.broadcast_to(
                np.array([swq, skv, swo, sbn, sbv], np.float32), (128, 5)).copy(),
            "wg3": wg3,
            "wu3": wu3,
            "wd3": wd3,
            "sg": sg2,
            "su": su2,
            "sd": sd2,
            "cosL": np.ascontiguousarray(cos_f[sl]),
            "sinL": np.ascontiguousarray(sin_f[sl]),
        })
    return in_maps


def run_cfg(cfg, nc, inputs_dict):
    from concourse.bass_utils import run_bass_kernel_spmd
    c = _derived(cfg)
    in_maps = prep_inputs(cfg, **inputs_dict)
    res = run_bass_kernel_spmd(nc, in_maps, list(range(cfg["N_CORES"])))
    out = np.concatenate(
        [res.results[i]["out"] for i in range(cfg["N_CORES"])], axis=0)
    return out.reshape(cfg["B"], cfg["S"], cfg["HID"]).astype(np.float32), res


_NC_CACHE = {}


def kernel(hidden_states, position_ids, Wq, Wkva, w_kvln, Wkvb, Wo, Wg, Wu, Wd,
           w_ln1, w_ln2):
    cfg = FULL_CFG
    if "full" not in _NC_CACHE:
        _NC_CACHE["full"] = build_kernel(cfg)
    out, _ = run_cfg(cfg, _NC_CACHE["full"], dict(
        hidden_states=np.asarray(hidden_states, np.float32),
        position_ids=np.asarray(position_ids),
        Wq=np.asarray(Wq, np.float32), Wkva=np.asarray(Wkva, np.float32),
        w_kvln=np.asarray(w_kvln, np.float32), Wkvb=np.asarray(Wkvb, np.float32),
        Wo=np.asarray(Wo, np.float32), Wg=np.asarray(Wg, np.float32),
        Wu=np.asarray(Wu, np.float32), Wd=np.asarray(Wd, np.float32),
        w_ln1=np.asarray(w_ln1, np.float32), w_ln2=np.asarray(w_ln2, np.float32)))
    return out

